# revision 22
# baseline (speedup 1.0000x reference)
"""DeepGCN ResNet (4-layer GCN w/ residuals + log_softmax) on 8 TRN2 NeuronCores.

Sharding: nodes split 8 ways; edges partitioned by destination row.
Per layer: local dense x@W (bf16) -> two fp8 AllGathers of h, split by
source-position bucket (A = rows < SPLIT per core, B = rest); the A-shard
collective launches mid-dense and A-bucket gathers overlap the B collective.
Source rows are fetched per edge with dma_gather (fp8, 256B descriptors,
single_packet, 4 SWDGE queues so all four Q7 descriptor-generation pairs run
concurrently), multiplied by HOST-precomputed fp8 one-hot tiles (DMA-loaded
from DRAM per window pair; the graph is layer-invariant so the tiles are
built once on host — this keeps the DVE out of the inner loop entirely) on
TensorE with PSUM accumulation per 128-dest window, then +res0
(SBUF-resident) and relu.
The res0/rlast projection itself runs under the layer-0 AllGather shadow.
Final layer adds res0@W_res_last and a batched log_softmax (one Ln pass).

Hard-won constraints (see memory notes): gather queue_num MUST equal the
scheduled DMASW sem-lane %4 (_build_aligned enforces this; the tile
scheduler reorders gathers); single_packet caps calls at 64 descs/engine
(<=5 tiles); pad gather indices with 0, never -1.

Host preprocessing is layout-only: greedy dest->window packing, edge
bucketing/sorting, int16 index packing (per-bucket index spaces), and the
per-tile (dest-row, val) scalar table for the DVE one-hot build.
"""

import numpy as np

import concourse.bacc as bacc
import concourse.mybir as mybir
import concourse.tile as tile
from concourse.bass import InstructionNameOrderedSet
from concourse.bass_utils import run_bass_kernel_spmd

P = 128

# Problem geometry (hardcoded per the task contract).
N_NODES = 50000
N_EDGES = 800000
F_IN = 256
HID = 256
C_OUT = 40
NCORES = 8

NLOC = N_NODES // NCORES            # 6250
NPAD = ((NLOC + P - 1) // P) * P    # 6272
W_WIN = NPAD // P                   # 49 windows of 128 dest rows
# Source rows are bucketed by their (permuted) position within the owner
# core: A = rows [0, SPLIT), B = rows [SPLIT, NPAD). Each bucket gets its own
# AllGather, so A-tile gathers can run while the B AllGather is in flight.
# Both bucket index spaces stay under int16 max.
SPLIT = 24 * P                      # 3072 (6 dense groups of 512)
ASH = SPLIT                         # A shard rows per core
BSH = NPAD - SPLIT                  # 3200 B shard rows per core


def _set_geometry(n_nodes):
    """Debug hook: shrink the node count (keeps F/HID/C). Used only by the
    small-scale simulator test, never in grading."""
    global N_NODES, NLOC, NPAD, W_WIN, SPLIT, ASH, BSH
    N_NODES = n_nodes
    NLOC = N_NODES // NCORES
    NPAD = ((NLOC + P - 1) // P) * P
    W_WIN = NPAD // P
    SPLIT = (W_WIN // 2) * P
    ASH = SPLIT
    BSH = NPAD - SPLIT

F32 = mybir.dt.float32
BF16 = mybir.dt.bfloat16
FP8 = mybir.dt.float8e4
I16 = mybir.dt.int16
MAX_GATHER_TILES = 999
import os as _os
# multi-packet descriptors (single_packet=False) measured ~870us faster once
# the gathers became the bottleneck; K_SP=1 restores the old mode
GATHER_SINGLE_PACKET = bool(_os.environ.get("K_SP"))
# prepare_only/trigger mode (K_PREP=1): measured SLOWER (3.42ms vs 2.28ms) --
# the framework does not defer a DRAM-source dep to the trigger, so the prep
# still waits on the AllGather and the added triggers/PE-waits are pure
# overhead. Kept for reference.
GATHER_PREP = bool(_os.environ.get("K_PREP"))
# fp8 DoubleRow: one PE instruction per two same-window edge tiles
DOUBLE_ROW = not _os.environ.get("K_NODBL")
SKIP_GATHER = bool(_os.environ.get("K_SKIP_GATHER"))   # bisect: omit dma_gather
SKIP_SPMM_MM = bool(_os.environ.get("K_SKIP_SPMM"))    # bisect: omit segment matmuls
SKIP_COLL = bool(_os.environ.get("K_SKIP_COLL"))       # bisect: omit AllGathers
SKIP_OH = bool(_os.environ.get("K_SKIP_OH"))           # bisect: omit oh tile loads
GATHER_QUEUES = int(_os.environ.get("K_GQ", "4"))  # rotate dma_gather queues
# sub-calls per gather call (desc-gen fixed-cost amortization); defaults to
# GATHER_QUEUES (legacy: one sub-call per queue)
GATHER_NSUB = int(_os.environ.get("K_NSUB", "0")) or GATHER_QUEUES


# ----------------------------------------------------------------- host prep


def preprocess(edge_row, edge_col, edge_val):
    """Edge partitioning/sorting/padding with per-core dest->window
    rebalancing (greedy 2D bin-pack on lo/hi in-degree). Returns per-core
    data arrays plus a core-uniform tile structure."""
    edge_row = edge_row.astype(np.int64)
    edge_col = edge_col.astype(np.int64)

    # --- lo/hi in-degree per destination (lo = src owned by cores 0..3)
    src_core0 = edge_col // NLOC
    src_hi0 = src_core0 >= (NCORES // 2)
    lo_deg = np.zeros(N_NODES, np.int64)
    hi_deg = np.zeros(N_NODES, np.int64)
    np.add.at(lo_deg, edge_row, ~src_hi0)
    np.add.at(hi_deg, edge_row, src_hi0)

    # --- per-core greedy assignment of dests to windows (balance lo & hi)
    pos_of_node = np.zeros(N_NODES, np.int64)
    perms = []  # per core: padded_pos[d_local]
    for c in range(NCORES):
        ld = lo_deg[c * NLOC:(c + 1) * NLOC].astype(np.float64)
        hd = hi_deg[c * NLOC:(c + 1) * NLOC].astype(np.float64)
        order = np.argsort(-(ld + hd), kind="stable")
        # windows 0..N_BIG-1 get a 9-tile budget (1152); rest hard-capped at
        # 1024 so they stay 8 tiles. Every core's excess lands in the same
        # window indices, so the cross-core max stays tight.
        N_BIG = 5
        cap = np.full(W_WIN, 8.0 * P)
        cap[:N_BIG] = 9.0 * P
        loads_lo = np.zeros(W_WIN)
        loads_hi = np.zeros(W_WIN)
        counts = np.zeros(W_WIN, np.int64)
        wassign = np.zeros(NLOC, np.int64)
        for t, d in enumerate(order):
            ccap = min(P, t // W_WIN + 2)  # stay within 2 of even fill
            cost = np.maximum(loads_lo + ld[d], loads_hi + hd[d])
            infeas = ((counts >= ccap) | (loads_lo + ld[d] > cap)
                      | (loads_hi + hd[d] > cap))
            if infeas.all():
                infeas = counts >= ccap
            if infeas.all():
                infeas = counts >= P
            cost = np.where(infeas, 1e18, cost)
            w = int(np.argmin(cost))
            wassign[d] = w
            loads_lo[w] += ld[d]
            loads_hi[w] += hd[d]
            counts[w] += 1
        # slot within window
        slot_in_w = np.zeros(NLOC, np.int64)
        fill = np.zeros(W_WIN, np.int64)
        for d in range(NLOC):
            w = wassign[d]
            slot_in_w[d] = fill[w]
            fill[w] += 1
        p = wassign * P + slot_in_w
        perms.append(p)
        pos_of_node[c * NLOC:(c + 1) * NLOC] = c * NPAD + p

    core = edge_row // NLOC
    p_local = pos_of_node[edge_row] - core * NPAD
    win = p_local // P
    d8 = p_local % P
    srcg = pos_of_node[edge_col]
    p_src = srcg % NPAD
    is_hi = (p_src >= SPLIT).astype(np.int64)  # B bucket

    lo_cnt = np.zeros((NCORES, W_WIN), np.int64)
    hi_cnt = np.zeros((NCORES, W_WIN), np.int64)
    np.add.at(lo_cnt, (core, win), 1 - is_hi)
    np.add.at(hi_cnt, (core, win), is_hi)

    lo_T = np.maximum(1, -(-lo_cnt.max(axis=0) // P))
    hi_T = -(-hi_cnt.max(axis=0) // P)
    win_T = lo_T + hi_T
    T_total = int(win_T.sum())

    # Paired-window stream: [wA_lo | wB_lo | wA_hi | wB_hi] so one gather
    # call covers both windows' lo (resp. hi) tiles -> fewer, bigger calls.
    lo_base = np.zeros(W_WIN, np.int64)
    hi_base = np.zeros(W_WIN, np.int64)
    pairs = [tuple(range(p, min(p + 2, W_WIN))) for p in range(0, W_WIN, 2)]
    gcalls = []   # per pair: [(tile_offset, n_tiles, is_hi), ...]
    spans = {}    # window -> [(tile_offset, n_tiles), ...] for matmuls
    t = 0
    for pr in pairs:
        t0 = t
        for w in pr:
            lo_base[w] = t * P
            spans[w] = [(t, int(lo_T[w]))]
            t += int(lo_T[w])
        calls = [(t0, t - t0, 0)]
        t1 = t
        for w in pr:
            hi_base[w] = t * P
            if hi_T[w] > 0:
                spans[w].append((t, int(hi_T[w])))
            t += int(hi_T[w])
        if t > t1:
            calls.append((t1, t - t1, 1))
        gcalls.append(calls)
    assert t == T_total
    runs = None  # superseded by pairs/gcalls/spans

    idx_all = np.zeros((NCORES, T_total * P), np.int16)
    oh_val = np.zeros((NCORES, T_total * P), np.float32)
    oh_d8 = np.zeros((NCORES, T_total * P), np.int64)  # pads stay d8=0,val=0

    order = np.lexsort((is_hi, win, core))
    e_core, e_win, e_hi = core[order], win[order], is_hi[order]
    e_srcg, e_d8, e_val = srcg[order], d8[order], edge_val[order]

    grp = (e_core * W_WIN + e_win) * 2 + e_hi
    cnt = np.zeros(NCORES * W_WIN * 2 + 1, np.int64)
    np.add.at(cnt, grp + 1, 1)
    starts = np.cumsum(cnt)[:-1]
    within = np.arange(len(order)) - starts[grp]

    slot = np.where(e_hi == 0, lo_base[e_win], hi_base[e_win]) + within
    e_sc = e_srcg // NPAD
    e_ps = e_srcg % NPAD
    newidx = np.where(e_hi == 0, e_sc * ASH + e_ps,
                      e_sc * BSH + (e_ps - SPLIT))
    idx_all[e_core, slot] = newidx.astype(np.int16)
    oh_val[e_core, slot] = e_val
    oh_d8[e_core, slot] = e_d8

    return dict(idx_all=idx_all, oh_val=oh_val, oh_d8=oh_d8, pairs=pairs,
                gcalls=gcalls, spans=spans, win_T=win_T, T_total=T_total,
                perms=perms)


def build_in_maps(inputs, pp):
    import ml_dtypes
    raw_x = np.ascontiguousarray(inputs["raw_x"], dtype=np.float32)
    T = pp["T_total"]

    def wblocks(w, fout_pad):
        wp = np.zeros((w.shape[0], fout_pad), np.float32)
        wp[:, :w.shape[1]] = w
        kh = w.shape[0] // P
        return wp.reshape(kh, P, fout_pad)

    W012 = np.stack([
        np.stack([np.stack([inputs[k][a * P:(a + 1) * P, b * P:(b + 1) * P]
                            for b in range(2)]) for a in range(2)])
        for k in ("W_gc0", "W_gc1", "W_gc2")]).astype(ml_dtypes.bfloat16)
    W3 = wblocks(np.asarray(inputs["W_gc3"], np.float32),
                 128).astype(ml_dtypes.bfloat16)
    Wr0 = np.stack([np.stack([inputs["W_res0"][a * P:(a + 1) * P, b * P:(b + 1) * P]
                              for b in range(2)]) for a in range(2)]
                   ).astype(ml_dtypes.bfloat16)
    WrL = wblocks(np.asarray(inputs["W_res_last"], np.float32), 64)
    ident = np.eye(P, dtype=np.float32)

    in_maps = []
    for c in range(NCORES):
        xT0 = np.zeros((2, P, NPAD), np.float32)
        xloc = raw_x[c * NLOC:(c + 1) * NLOC]          # [NLOC, 256]
        xT0[:, :, pp["perms"][c]] = xloc.T.reshape(2, P, NLOC)
        xT0 = xT0.astype(ml_dtypes.bfloat16)

        idx16 = np.zeros((16, T * 8), np.int16)
        i = np.arange(T * P)
        idx16[i % 16, i // 16] = (
            (np.arange(T * P) % (NCORES * ASH)).astype(np.int16)
            if _os.environ.get("K_IDXSEQ") else pp["idx_all"][c])
        gidx = np.tile(idx16, (8, 1))                  # [128, T*8]

        # host-built one-hot tiles: oh[e, t, d] = val * (d == dest row), in
        # gather-slot-major order; layer-invariant, so built once and fp8.
        oh = np.zeros((T * P, P), np.float32)
        oh[np.arange(T * P), pp["oh_d8"][c]] = pp["oh_val"][c]
        ohtiles = np.ascontiguousarray(
            oh.reshape(T, P, P).transpose(1, 0, 2)).astype(
                ml_dtypes.float8_e4m3)

        in_maps.append(dict(xT0=xT0, gidx=gidx, ohtiles=ohtiles, W012=W012,
                            W3=W3, Wr0=Wr0, WrL=WrL, ident=ident))
    return in_maps


# ------------------------------------------------------------- device program


def build_program(pp, qmap=None):
    T = pp["T_total"]
    win_T = pp["win_T"]
    pairs, gcalls, spans = pp["pairs"], pp["gcalls"], pp["spans"]
    Tmax = max(sum(int(win_T[w]) for w in pr) for pr in pairs)
    NG = [(g * 512, min(512, NPAD - g * 512)) for g in range(-(-NPAD // 512))]

    nc = bacc.Bacc("TRN2", target_bir_lowering=False, debug=False,
                   num_devices=NCORES, num_swdge_queues=GATHER_QUEUES)

    xT0_d = nc.dram_tensor("xT0", [2, P, NPAD], BF16, kind="ExternalInput")
    gidx_d = nc.dram_tensor("gidx", [P, T * 8], I16, kind="ExternalInput")
    oh_d = nc.dram_tensor("ohtiles", [P, T, P], FP8, kind="ExternalInput")
    W012_d = nc.dram_tensor("W012", [3, 2, 2, P, P], BF16, kind="ExternalInput")
    W3_d = nc.dram_tensor("W3", [2, P, 128], BF16, kind="ExternalInput")
    Wr0_d = nc.dram_tensor("Wr0", [2, 2, P, P], BF16, kind="ExternalInput")
    WrL_d = nc.dram_tensor("WrL", [2, P, 64], F32, kind="ExternalInput")
    ident_d = nc.dram_tensor("ident", [P, P], F32, kind="ExternalInput")
    out_d = nc.dram_tensor("out", [NPAD, C_OUT], F32, kind="ExternalOutput")

    ag_inA = [nc.dram_tensor(f"ag_inA{i}", [ASH, HID], FP8) for i in range(2)]
    ag_inB = [nc.dram_tensor(f"ag_inB{i}", [BSH, HID], FP8) for i in range(2)]
    hfullA = [nc.dram_tensor(f"hfullA{i}", [NCORES * ASH, HID], FP8,
                             addr_space="Shared") for i in range(2)]
    hfullB = [nc.dram_tensor(f"hfullB{i}", [NCORES * BSH, HID], FP8,
                             addr_space="Shared") for i in range(2)]
    ag3A = nc.dram_tensor("ag3A", [ASH, 128], BF16)
    ag3B = nc.dram_tensor("ag3B", [BSH, 128], BF16)
    h3A = nc.dram_tensor("h3A", [NCORES * ASH, 128], BF16,
                         addr_space="Shared")
    h3B = nc.dram_tensor("h3B", [NCORES * BSH, 128], BF16,
                         addr_space="Shared")

    with tile.TileContext(nc) as tc:
        with (
            tc.tile_pool(name="pers", bufs=1) as pers,
            tc.tile_pool(name="wbufp", bufs=4) as wbufp,
            tc.tile_pool(name="ohp", bufs=4) as ohp,
            tc.tile_pool(name="hTs", bufs=3) as hTsp,
            tc.tile_pool(name="hns", bufs=4) as hnsp,
            tc.tile_pool(name="zs", bufs=4) as zsp,
            tc.tile_pool(name="r0s", bufs=4) as r0sp,
            tc.tile_pool(name="sm", bufs=6) as smp,
            tc.tile_pool(name="ps_s", bufs=4, space="PSUM") as ps_s,
            tc.tile_pool(name="ps_d", bufs=2, space="PSUM") as ps_d,
            tc.tile_pool(name="ps_t", bufs=2, space="PSUM") as ps_t,
        ):
            # ---------------- persistent loads
            xT = pers.tile([P, 2, NPAD], BF16, tag="xT")
            nc.sync.dma_start(out=xT[:, 0, :], in_=xT0_d.ap()[0])
            nc.sync.dma_start(out=xT[:, 1, :], in_=xT0_d.ap()[1])
            idxs = pers.tile([P, T * 8], I16, tag="gidx")
            nc.sync.dma_start(out=idxs[:], in_=gidx_d.ap())
            w012 = pers.tile([P, 12, P], BF16, tag="w012")
            nc.sync.dma_start(
                out=w012[:],
                in_=W012_d.ap().rearrange("a b c p m -> p (a b c) m"))
            w3 = pers.tile([P, 2, 128], BF16, tag="w3")
            nc.sync.dma_start(out=w3[:], in_=W3_d.ap().rearrange("a p m -> p a m"))
            wr0 = pers.tile([P, 4, P], BF16, tag="wr0")
            nc.sync.dma_start(out=wr0[:],
                              in_=Wr0_d.ap().rearrange("a b p m -> p (a b) m"))
            wrl = pers.tile([P, 2, 64], F32, tag="wrl")
            nc.sync.dma_start(out=wrl[:], in_=WrL_d.ap().rearrange("a p m -> p a m"))
            ident = pers.tile([P, P], F32, tag="ident")
            nc.sync.dma_start(out=ident[:], in_=ident_d.ap())
            # SBUF-resident residuals (window-indexed) + softmax staging
            res0_sb = pers.tile([P, W_WIN, HID], F32, tag="res0sb")
            rlast_sb = pers.tile([P, W_WIN, 64], F32, tag="rlastsb")
            tt_sb = pers.tile([P, W_WIN, C_OUT], F32, tag="ttsb")
            ssum_sb = pers.tile([P, W_WIN], F32, tag="ssumsb")

            def w012_ap(li, kh, fh):
                return w012[:, li * 4 + kh * 2 + fh, :]

            # ------------- dense helper: h[:, fh*128:...] = x @ W  (+ hooks)
            def dense256(w_ap_fn, dest_dram, rl_hook=None, hn_dt=BF16,
                         hn_tag="hn", mid_hook=None, dest_sb=None):
                """w_ap_fn(kh, fh) -> lhsT [128,128]. Writes node-major
                rows to dest_dram=(destA, destB) split at row SPLIT, or into
                dest_sb[:, w, :] (SBUF, window-indexed). mid_hook() runs
                after the last A group -- launches the A-shard AllGather."""
                destA, destB = (dest_dram if isinstance(dest_dram, tuple)
                                else (dest_dram, None))
                for g0, ng in NG:
                    stages = []
                    for fh in range(2):
                        psd = ps_d.tile([P, 512], F32, space="PSUM", tag="dps")
                        for kh in range(2):
                            nc.tensor.matmul(
                                psd[:, :ng], lhsT=w_ap_fn(kh, fh),
                                rhs=xT[:, kh, g0:g0 + ng],
                                start=(kh == 0), stop=(kh == 1))
                        hTst = hTsp.tile([P, 512], F32, tag="hT")
                        nc.vector.tensor_copy(out=hTst[:, :ng], in_=psd[:, :ng])
                        stages.append(hTst)
                        if rl_hook is not None:
                            rl_hook(fh, g0, ng, hTst)
                    for s in range(ng // P):
                        w = g0 // P + s
                        hn = (None if dest_sb is not None else
                              hnsp.tile([P, HID], hn_dt, tag=hn_tag))
                        for fh in range(2):
                            pst = ps_t.tile([P, P], F32, space="PSUM", tag="tp")
                            nc.tensor.transpose(
                                out=pst[:],
                                in_=stages[fh][:, s * P:(s + 1) * P],
                                identity=ident[:])
                            nc.vector.tensor_copy(
                                out=(dest_sb[:, w, fh * P:(fh + 1) * P]
                                     if dest_sb is not None
                                     else hn[:, fh * P:(fh + 1) * P]),
                                in_=pst[:])
                        if dest_sb is None:
                            r0 = g0 + s * P
                            dst = (destA.ap()[r0:r0 + P, :] if destB is None
                                   or r0 < SPLIT else
                                   destB.ap()[r0 - SPLIT:r0 - SPLIT + P, :])
                            nc.sync.dma_start(out=dst, in_=hn[:])
                    if mid_hook is not None and g0 + ng == SPLIT:
                        mid_hook()

            # ---------------- init: res0 (+ rlast) from raw_x
            rl_ps = {}

            def rl_hook(fh, g0, ng, hTst):
                if fh == 0:
                    rl_ps["t"] = ps_d.tile([P, 512], F32, space="PSUM",
                                           tag="dps", name="psr")
                psr = rl_ps["t"]
                nc.tensor.matmul(psr[:64, :ng], lhsT=wrl[:, fh, :],
                                 rhs=hTst[:, :ng],
                                 start=(fh == 0), stop=(fh == 1))
                if fh == 1:
                    rlT = hTsp.tile([P, 512], F32, tag="hT")
                    nc.vector.tensor_copy(out=rlT[:64, :ng], in_=psr[:64, :ng])
                    for s in range(ng // P):
                        w = g0 // P + s
                        pst = ps_t.tile([P, P], F32, space="PSUM", tag="tp")
                        nc.tensor.transpose(out=pst[:, :64],
                                            in_=rlT[:64, s * P:(s + 1) * P],
                                            identity=ident[:64, :64])
                        nc.vector.tensor_copy(out=rlast_sb[:, w, :],
                                              in_=pst[:, :64])

            # (res0/rlast dense is issued inside the layer-0 region below,
            # after the layer-0 AllGathers launch, so it runs while they fly)

            # ---------------- spmm helper (paired windows)
            qrr = [0]  # round-robin SWDGE queue counter
            gsems = [nc.alloc_semaphore(f"gsem{q}")
                     for q in range(GATHER_QUEUES)] if GATHER_PREP else None
            last_trig = [None] * GATHER_QUEUES
            prep_cnt = [0] * GATHER_QUEUES  # completed-DMA sem targets (/16)
            gnames = []  # emission-ordered gather instruction names
            owner = {}
            for w, sp in spans.items():
                for (t0, nt) in sp:
                    for t in range(t0, t0 + nt):
                        owner[t] = w

            def spmm(hfA, hfB, elem, psw, evict_fn, gdt=BF16):
                for ip, pr in enumerate(pairs):
                    p0 = spans[pr[0]][0][0]
                    nTp = sum(int(win_T[w]) for w in pr)
                    ohb = ohp.tile([P, Tmax, P], FP8, tag="ohb")
                    if SKIP_OH:
                        nc.vector.memset(ohb[:, 0, :], 0.0)
                    else:
                        # host-precomputed one-hot tiles; split the load so
                        # consecutive halves land on different DMA lanes
                        h = (nTp + 1) // 2
                        nc.sync.dma_start(out=ohb[:, 0:h, :],
                                          in_=oh_d.ap()[:, p0:p0 + h, :])
                        nc.sync.dma_start(
                            out=ohb[:, h:nTp, :],
                            in_=oh_d.ap()[:, p0 + h:p0 + nTp, :])
                    wbg = wbufp.tile([P, Tmax, elem], gdt, tag="wbg")
                    if SKIP_GATHER:
                        nc.vector.memset(wbg[:, 0, :], 0.0)
                    gwaits = {}  # tile idx -> [(sem, target, wait_names)]
                    for (c0, cn, hi) in gcalls[ip]:
                        if SKIP_GATHER:
                            continue
                        src = (hfB.ap() if hi else hfA.ap())
                        # one sub-call per SWDGE queue so each call-group's
                        # transfer drains through all four queue FIFOs
                        nsub = min(GATHER_NSUB, cn) or 1
                        bounds = [c0 + (cn * k) // nsub for k in range(nsub + 1)]
                        qs = set()
                        for k in range(nsub):
                            t0, t1 = bounds[k], bounds[k + 1]
                            nt = t1 - t0
                            if nt == 0:
                                continue
                            gi = qrr[0]
                            if GATHER_PREP:
                                q = gi % GATHER_QUEUES
                                # prep writes only descriptors; the transfer
                                # fires at the trigger below and completion
                                # bumps gsems[q] by 16
                                gin = nc.gpsimd.dma_gather(
                                    wbg[:, t0 - p0:t0 - p0 + nt, :], src,
                                    idxs[:, t0 * 8:(t0 + nt) * 8],
                                    nt * P, nt * P, elem,
                                    single_packet=GATHER_SINGLE_PACKET,
                                    queue_num=q, prepare_only=True,
                                    sem=gsems[q])
                                # pin ring order: a prep may not be scheduled
                                # across an earlier trigger of its queue, else
                                # that trigger would fire this prep's entries
                                if last_trig[q] is not None:
                                    dep = InstructionNameOrderedSet()
                                    dep.add(last_trig[q])
                                    gin.ins.add_nosync_dependencies_from(dep)
                                prep_cnt[q] += 1
                                qs.add(q)
                            else:
                                # queue must equal (scheduled DMASW lane) % 4;
                                # the scheduler can reorder gathers, so a
                                # corrective qmap takes precedence
                                q = (qmap[gi] if qmap is not None
                                     and gi < len(qmap)
                                     else gi % GATHER_QUEUES)
                                gin = nc.gpsimd.dma_gather(
                                    wbg[:, t0 - p0:t0 - p0 + nt, :], src,
                                    idxs[:, t0 * 8:(t0 + nt) * 8],
                                    nt * P, nt * P, elem,
                                    single_packet=GATHER_SINGLE_PACKET,
                                    queue_num=q)
                            gnames.append(gin.ins.name)
                            qrr[0] += 1
                        for q in sorted(qs):
                            trig = nc.gpsimd.trigger_dma(count=None,
                                                         queue_num=q)
                            last_trig[q] = trig.ins.name
                        if GATHER_PREP:
                            # PE must wait for this call-group's data before
                            # consuming tiles [c0, c0+cn)
                            gwaits[c0] = [(gsems[q], prep_cnt[q] * 16)
                                          for q in sorted(qs)]
                    # fp8 gather output feeds the matmul directly (mixed
                    # bf16 lhsT x fp8 rhs); when both operands are fp8,
                    # DoubleRow mode folds two consecutive same-window tiles
                    # into one PE instruction (256-deep contraction)
                    wb = wbg
                    dbl = DOUBLE_ROW and gdt == FP8 and not SKIP_SPMM_MM
                    psl = {w: ps_s.tile([P, HID], F32, space="PSUM",
                                        tag="sps", name=f"pss_{ip}_{w}")
                           for w in pr}
                    cnt = {w: 0 for w in pr}
                    wait_names = []

                    def mm(w, t, take):
                        tot = int(win_T[w])
                        if SKIP_SPMM_MM and not (cnt[w] == 0 or
                                                 cnt[w] + take >= tot):
                            cnt[w] += take
                            return
                        if take == 2:
                            mi = nc.tensor.matmul(
                                psl[w][:, :psw],
                                lhsT=ohb[:, t - p0:t - p0 + 2, :],
                                rhs=wb[:, t - p0:t - p0 + 2, :],
                                start=(cnt[w] == 0),
                                stop=(cnt[w] + 2 == tot),
                                perf_mode=mybir.MatmulPerfMode.DoubleRow)
                        else:
                            mi = nc.tensor.matmul(
                                psl[w][:, :psw],
                                lhsT=ohb[:, t - p0, :],
                                rhs=wb[:, t - p0, :],
                                start=(cnt[w] == 0),
                                stop=(cnt[w] + 1 == tot))
                        if wait_names:
                            # keep every consumer behind its group's PE waits
                            # (PE executes in order; pin the schedule order)
                            dep = InstructionNameOrderedSet()
                            for wn in wait_names:
                                dep.add(wn)
                            mi.ins.add_nosync_dependencies_from(dep)
                        cnt[w] += take

                    t = p0
                    while t < p0 + nTp:
                        w = owner[t]
                        if t in gwaits:
                            wait_names = []
                            for sem, tgt in gwaits[t]:
                                wi = nc.tensor.wait_ge(sem, tgt)
                                wait_names.append(wi.ins.name)
                        take = (2 if dbl and t + 1 < p0 + nTp
                                and owner[t + 1] == w and t + 1 not in gwaits
                                else 1)
                        mm(w, t, take)
                        t += take
                    for w in pr:
                        evict_fn(w, psl[w])

            # ---------------- GCN layers 0..2
            rg = [list(range(NCORES))]

            def ag(in_d, out_d):
                if not SKIP_COLL:
                    nc.gpsimd.collective_compute(
                        "AllGather", mybir.AluOpType.bypass,
                        replica_groups=rg, ins=[in_d.ap()], outs=[out_d.ap()])

            for li in range(3):
                pa = li % 2
                dense256(lambda kh, fh, li=li: w012_ap(li, kh, fh),
                         (ag_inA[pa], ag_inB[pa]), hn_dt=FP8,
                         mid_hook=lambda pa=pa: ag(ag_inA[pa], hfullA[pa]))
                ag(ag_inB[pa], hfullB[pa])
                if li == 0:
                    # res0/rlast dense overlaps the layer-0 AllGathers (it
                    # must finish before the first evict overwrites xT; the
                    # tile deps enforce that)
                    dense256(lambda kh, fh: wr0[:, kh * 2 + fh, :], None,
                             rl_hook=rl_hook, dest_sb=res0_sb)

                def evict_gc(w, pss):
                    z = zsp.tile([P, HID], F32, tag="z")
                    nc.vector.tensor_add(out=z[:], in0=pss[:],
                                         in1=res0_sb[:, w, :])
                    for fh in range(2):
                        pst = ps_t.tile([P, P], F32, space="PSUM", tag="tp")
                        nc.tensor.transpose(out=pst[:],
                                            in_=z[:, fh * P:(fh + 1) * P],
                                            identity=ident[:])
                        nc.scalar.activation(
                            out=xT[:, fh, w * P:(w + 1) * P], in_=pst[:],
                            func=mybir.ActivationFunctionType.Relu)

                spmm(hfullA[pa], hfullB[pa], HID, HID, evict_gc, gdt=FP8)

            # ---------------- layer 3 dense (fout=64 incl. padding)
            for g0, ng in NG:
                psd = ps_d.tile([P, 512], F32, space="PSUM", tag="dps")
                for kh in range(2):
                    nc.tensor.matmul(psd[:, :ng], lhsT=w3[:, kh, :],
                                     rhs=xT[:, kh, g0:g0 + ng],
                                     start=(kh == 0), stop=(kh == 1))
                h3T = hTsp.tile([P, 512], F32, tag="hT")
                nc.vector.tensor_copy(out=h3T[:, :ng], in_=psd[:, :ng])
                for s in range(ng // P):
                    pst = ps_t.tile([P, P], F32, space="PSUM", tag="tp")
                    nc.tensor.transpose(out=pst[:],
                                        in_=h3T[:, s * P:(s + 1) * P],
                                        identity=ident[:])
                    h3n = hnsp.tile([P, 128], BF16, tag="h3n")
                    nc.vector.tensor_copy(out=h3n[:], in_=pst[:])
                    r0 = g0 + s * P
                    dst3 = (ag3A.ap()[r0:r0 + P, :] if r0 < SPLIT
                            else ag3B.ap()[r0 - SPLIT:r0 - SPLIT + P, :])
                    nc.sync.dma_start(out=dst3, in_=h3n[:])
                if g0 + ng == SPLIT:
                    ag(ag3A, h3A)
            ag(ag3B, h3B)

            # ---------------- layer 3 spmm + rlast + log_softmax
            # Exp runs per window (accumulating row sums into ssum_sb); Ln
            # runs ONCE over [P, W_WIN] at the end -- no per-window
            # activation-table reloads.
            def evict_out(w, pss):
                y = zsp.tile([P, 64], F32, tag="y")
                nc.vector.tensor_add(out=y[:], in0=pss[:, :64],
                                     in1=rlast_sb[:, w, :])
                m = smp.tile([P, 1], F32, tag="m")
                nc.vector.tensor_reduce(out=m[:], in_=y[:, :C_OUT],
                                        axis=mybir.AxisListType.X,
                                        op=mybir.AluOpType.max)
                nc.vector.tensor_scalar(out=tt_sb[:, w, :], in0=y[:, :C_OUT],
                                        scalar1=m[:], scalar2=None,
                                        op0=mybir.AluOpType.subtract)
                e = smp.tile([P, C_OUT], F32, tag="e")
                nc.scalar.activation(out=e[:], in_=tt_sb[:, w, :],
                                     func=mybir.ActivationFunctionType.Exp,
                                     accum_out=ssum_sb[:, w:w + 1])

            spmm(h3A, h3B, 128, 128, evict_out)

            lgs = pers.tile([P, W_WIN], F32, tag="lgs")
            nc.scalar.activation(out=lgs[:], in_=ssum_sb[:],
                                 func=mybir.ActivationFunctionType.Ln)
            for w in range(W_WIN):
                o = smp.tile([P, C_OUT], F32, tag="o")
                nc.vector.tensor_scalar(out=o[:], in0=tt_sb[:, w, :],
                                        scalar1=lgs[:, w:w + 1], scalar2=None,
                                        op0=mybir.AluOpType.subtract)
                nc.sync.dma_start(out=out_d.ap()[w * P:(w + 1) * P, :],
                                  in_=o[:])

    nc.compile()
    nc._gather_names = gnames
    return nc


def _gather_qmap(nc):
    """Walk the scheduled program: gather g's DMASW sem lane is its
    scheduled Pool-DMA index %8, so its queue_num must be (lane %
    GATHER_QUEUES) or cross-queue sem races ensue. Returns (qmap keyed by
    emission order, already_aligned)."""
    import concourse.bass_isa as bass_isa
    sched_q = {}
    emitted_q = {}
    k = 0
    for blk in nc.m.functions[0].blocks:
        for inst in blk.instructions:
            if (inst.engine == mybir.EngineType.Pool
                    and isinstance(inst, bass_isa.AnyDMAInstruction)
                    and not isinstance(inst,
                                       bass_isa.UserSyncedRemoteDMADescs)):
                sched_q[inst.name] = (k % 8) % GATHER_QUEUES
                emitted_q[inst.name] = inst.queue_num
                k += 1
    qmap = [sched_q[nm] for nm in nc._gather_names]
    ok = all(sched_q[nm] == emitted_q[nm] for nm in nc._gather_names)
    return qmap, ok


def _build_aligned(pp):
    """Build, then realign gather queue_nums to the scheduled sem lanes
    (the tile scheduler may reorder gathers); iterate to a fixpoint.
    In GATHER_PREP mode each queue has its own explicit DMA semaphore, so
    no realignment is needed."""
    nc = build_program(pp)
    if GATHER_PREP:
        return nc
    for _ in range(3):
        qmap, ok = _gather_qmap(nc)
        if ok:
            return nc
        nc = build_program(pp, qmap)
    qmap, ok = _gather_qmap(nc)
    assert ok, "gather queue/sem-lane alignment did not converge"
    return nc


# ------------------------------------------------------------ timed benchmark


def bench(nc, in_maps, iters=8):
    """Times repeated on-device executions with device-resident inputs
    (replicates bass2jax.run_bass_via_pjrt's multi-core path). Returns
    (best_seconds, per_iter_seconds)."""
    import time

    import jax
    from jax.experimental.shard_map import shard_map
    from jax.sharding import Mesh, NamedSharding, PartitionSpec

    from concourse import bass2jax, mybir as mb

    bass2jax.install_neuronx_cc_hook()

    partition_name = (nc.partition_id_tensor.name
                      if nc.partition_id_tensor else None)
    in_names, out_names, out_avals, zero_outs = [], [], [], []
    for alloc in nc.m.functions[0].allocations:
        if not isinstance(alloc, mb.MemoryLocationSet):
            continue
        name = alloc.memorylocations[0].name
        if alloc.kind == "ExternalInput":
            if name != partition_name:
                in_names.append(name)
        elif alloc.kind == "ExternalOutput":
            out_names.append(name)
            shape = tuple(alloc.tensor_shape)
            dtype = mb.dt.np(alloc.dtype)
            out_avals.append(jax.core.ShapedArray(shape, dtype))
            zero_outs.append(np.zeros(shape, dtype))
    n_params = len(in_names)
    n_outs = len(out_avals)
    all_names = in_names + out_names
    if partition_name is not None:
        all_names = all_names + [partition_name]

    def _body(*args):
        operands = list(args)
        if partition_name is not None:
            operands.append(bass2jax.partition_id_tensor())
        outs = bass2jax._bass_exec_p.bind(
            *operands, out_avals=tuple(out_avals), in_names=tuple(all_names),
            out_names=tuple(out_names), lowering_input_output_aliases=(),
            sim_require_finite=True, sim_require_nnan=True, nc=nc)
        return tuple(outs)

    devices = jax.devices()[:NCORES]
    mesh = Mesh(np.asarray(devices), ("core",))
    in_specs = (PartitionSpec("core"),) * (n_params + n_outs)
    out_specs = (PartitionSpec("core"),) * n_outs
    donate = tuple(range(n_params, n_params + n_outs))
    del donate
    sharded = jax.jit(shard_map(_body, mesh=mesh, in_specs=in_specs,
                                out_specs=out_specs, check_rep=False),
                      keep_unused=True)

    sh = NamedSharding(mesh, PartitionSpec("core"))
    dev_in = [
        jax.device_put(
            np.concatenate([np.asarray(in_maps[c][n]) for c in range(NCORES)],
                           axis=0), sh)
        for n in in_names]
    zglobal = [np.zeros((NCORES * z.shape[0], *z.shape[1:]), z.dtype)
               for z in zero_outs]

    dz = [jax.device_put(z, sh) for z in zglobal]
    for d in dz:
        d.block_until_ready()

    def run_batch(n):
        """Queue n executions without intermediate sync; return elapsed."""
        t0 = time.perf_counter()
        outs = None
        for _ in range(n):
            outs = sharded(*dev_in, *dz)
        for o in outs:
            o.block_until_ready()
        return time.perf_counter() - t0

    run_batch(1)  # warmup
    n_hi = 41
    lo = min(run_batch(1) for _ in range(iters))
    hi = min(run_batch(n_hi) for _ in range(max(2, iters // 2)))
    per_exec = (hi - lo) / (n_hi - 1)
    return per_exec, (lo, hi)


# ---------------------------------------------------------------- entry point

_CACHE = {}


def _run(inputs, trace=False, trace_kwargs=None):
    pp = preprocess(np.asarray(inputs["edge_row"]),
                    np.asarray(inputs["edge_col"]),
                    np.asarray(inputs["edge_val"], dtype=np.float32))
    in_maps = build_in_maps(inputs, pp)
    key = (pp["T_total"], tuple(tuple(c) for cs in pp["gcalls"] for c in cs))
    if key not in _CACHE:
        _CACHE.clear()
        _CACHE[key] = _build_aligned(pp)
    nc = _CACHE[key]
    res = run_bass_kernel_spmd(nc, in_maps, list(range(NCORES)), trace=trace,
                               **(trace_kwargs or {}))
    outs = [res.results[c]["out"][pp["perms"][c]] for c in range(NCORES)]
    full = np.concatenate(outs, axis=0).astype(np.float32)
    return full, res


def kernel(**inputs):
    out, _ = _run(inputs)
    return out



# revision 23
# speedup vs baseline: 1.0180x; 1.0180x over previous
"""DeepGCN ResNet (4-layer GCN w/ residuals + log_softmax) on 8 TRN2 NeuronCores.

Sharding: nodes split 8 ways; edges partitioned by destination row.
Per layer: local dense x@W (bf16) -> two fp8 AllGathers of h, split by
source-position bucket (A = rows < SPLIT per core, B = rest); the A-shard
collective launches mid-dense and A-bucket gathers overlap the B collective.
Source rows are fetched per edge with dma_gather (fp8, 256B descriptors,
single_packet, 4 SWDGE queues so all four Q7 descriptor-generation pairs run
concurrently), multiplied by HOST-precomputed fp8 one-hot tiles (DMA-loaded
from DRAM per window pair; the graph is layer-invariant so the tiles are
built once on host — this keeps the DVE out of the inner loop entirely) on
TensorE with PSUM accumulation per 128-dest window, then +res0
(SBUF-resident) and relu.
The res0/rlast projection itself runs under the layer-0 AllGather shadow.
Final layer adds res0@W_res_last and a batched log_softmax (one Ln pass).

Hard-won constraints (see memory notes): gather queue_num MUST equal the
scheduled DMASW sem-lane %4 (_build_aligned enforces this; the tile
scheduler reorders gathers); single_packet caps calls at 64 descs/engine
(<=5 tiles); pad gather indices with 0, never -1.

Host preprocessing is layout-only: greedy dest->window packing, edge
bucketing/sorting, int16 index packing (per-bucket index spaces), and the
per-tile (dest-row, val) scalar table for the DVE one-hot build.
"""

import numpy as np

import concourse.bacc as bacc
import concourse.mybir as mybir
import concourse.tile as tile
from concourse.bass import InstructionNameOrderedSet
from concourse.bass_utils import run_bass_kernel_spmd

P = 128

# Problem geometry (hardcoded per the task contract).
N_NODES = 50000
N_EDGES = 800000
F_IN = 256
HID = 256
C_OUT = 40
NCORES = 8

NLOC = N_NODES // NCORES            # 6250
NPAD = ((NLOC + P - 1) // P) * P    # 6272
W_WIN = NPAD // P                   # 49 windows of 128 dest rows
# Source rows are bucketed by their (permuted) position within the owner
# core: A = rows [0, SPLIT), B = rows [SPLIT, NPAD). Each bucket gets its own
# AllGather, so A-tile gathers can run while the B AllGather is in flight.
# Both bucket index spaces stay under int16 max.
SPLIT = 24 * P                      # 3072 (6 dense groups of 512)
ASH = SPLIT                         # A shard rows per core
BSH = NPAD - SPLIT                  # 3200 B shard rows per core


def _set_geometry(n_nodes):
    """Debug hook: shrink the node count (keeps F/HID/C). Used only by the
    small-scale simulator test, never in grading."""
    global N_NODES, NLOC, NPAD, W_WIN, SPLIT, ASH, BSH
    N_NODES = n_nodes
    NLOC = N_NODES // NCORES
    NPAD = ((NLOC + P - 1) // P) * P
    W_WIN = NPAD // P
    SPLIT = (W_WIN // 2) * P
    ASH = SPLIT
    BSH = NPAD - SPLIT

F32 = mybir.dt.float32
BF16 = mybir.dt.bfloat16
FP8 = mybir.dt.float8e4
I16 = mybir.dt.int16
MAX_GATHER_TILES = 999
import os as _os
# multi-packet descriptors (single_packet=False) measured ~870us faster once
# the gathers became the bottleneck; K_SP=1 restores the old mode
GATHER_SINGLE_PACKET = bool(_os.environ.get("K_SP"))
# prepare_only/trigger mode (K_PREP=1): measured SLOWER (3.42ms vs 2.28ms) --
# the framework does not defer a DRAM-source dep to the trigger, so the prep
# still waits on the AllGather and the added triggers/PE-waits are pure
# overhead. Kept for reference.
GATHER_PREP = bool(_os.environ.get("K_PREP"))
# fp8 DoubleRow: one PE instruction per two same-window edge tiles
DOUBLE_ROW = not _os.environ.get("K_NODBL")
SKIP_GATHER = bool(_os.environ.get("K_SKIP_GATHER"))   # bisect: omit dma_gather
SKIP_SPMM_MM = bool(_os.environ.get("K_SKIP_SPMM"))    # bisect: omit segment matmuls
SKIP_COLL = bool(_os.environ.get("K_SKIP_COLL"))       # bisect: omit AllGathers
SKIP_OH = bool(_os.environ.get("K_SKIP_OH"))           # bisect: omit oh tile loads
GATHER_QUEUES = int(_os.environ.get("K_GQ", "4"))  # rotate dma_gather queues
# sub-calls per gather call (desc-gen fixed-cost amortization); defaults to
# GATHER_QUEUES (legacy: one sub-call per queue)
GATHER_NSUB = int(_os.environ.get("K_NSUB", "0")) or GATHER_QUEUES


# ----------------------------------------------------------------- host prep


def preprocess(edge_row, edge_col, edge_val):
    """Edge partitioning/sorting/padding with per-core dest->window
    rebalancing (greedy 2D bin-pack on lo/hi in-degree). Returns per-core
    data arrays plus a core-uniform tile structure."""
    edge_row = edge_row.astype(np.int64)
    edge_col = edge_col.astype(np.int64)

    # --- lo/hi in-degree per destination (lo = src owned by cores 0..3)
    src_core0 = edge_col // NLOC
    src_hi0 = src_core0 >= (NCORES // 2)
    lo_deg = np.zeros(N_NODES, np.int64)
    hi_deg = np.zeros(N_NODES, np.int64)
    np.add.at(lo_deg, edge_row, ~src_hi0)
    np.add.at(hi_deg, edge_row, src_hi0)

    # --- per-core greedy assignment of dests to windows (balance lo & hi)
    pos_of_node = np.zeros(N_NODES, np.int64)
    perms = []  # per core: padded_pos[d_local]
    for c in range(NCORES):
        ld = lo_deg[c * NLOC:(c + 1) * NLOC].astype(np.float64)
        hd = hi_deg[c * NLOC:(c + 1) * NLOC].astype(np.float64)
        order = np.argsort(-(ld + hd), kind="stable")
        # windows 0..N_BIG-1 get a 9-tile budget (1152); rest hard-capped at
        # 1024 so they stay 8 tiles. Every core's excess lands in the same
        # window indices, so the cross-core max stays tight.
        N_BIG = 5
        cap = np.full(W_WIN, 8.0 * P)
        cap[:N_BIG] = 9.0 * P
        loads_lo = np.zeros(W_WIN)
        loads_hi = np.zeros(W_WIN)
        counts = np.zeros(W_WIN, np.int64)
        wassign = np.zeros(NLOC, np.int64)
        for t, d in enumerate(order):
            ccap = min(P, t // W_WIN + 2)  # stay within 2 of even fill
            cost = np.maximum(loads_lo + ld[d], loads_hi + hd[d])
            infeas = ((counts >= ccap) | (loads_lo + ld[d] > cap)
                      | (loads_hi + hd[d] > cap))
            if infeas.all():
                infeas = counts >= ccap
            if infeas.all():
                infeas = counts >= P
            cost = np.where(infeas, 1e18, cost)
            w = int(np.argmin(cost))
            wassign[d] = w
            loads_lo[w] += ld[d]
            loads_hi[w] += hd[d]
            counts[w] += 1
        # slot within window
        slot_in_w = np.zeros(NLOC, np.int64)
        fill = np.zeros(W_WIN, np.int64)
        for d in range(NLOC):
            w = wassign[d]
            slot_in_w[d] = fill[w]
            fill[w] += 1
        p = wassign * P + slot_in_w
        perms.append(p)
        pos_of_node[c * NLOC:(c + 1) * NLOC] = c * NPAD + p

    core = edge_row // NLOC
    p_local = pos_of_node[edge_row] - core * NPAD
    win = p_local // P
    d8 = p_local % P
    srcg = pos_of_node[edge_col]
    p_src = srcg % NPAD
    is_hi = (p_src >= SPLIT).astype(np.int64)  # B bucket

    lo_cnt = np.zeros((NCORES, W_WIN), np.int64)
    hi_cnt = np.zeros((NCORES, W_WIN), np.int64)
    np.add.at(lo_cnt, (core, win), 1 - is_hi)
    np.add.at(hi_cnt, (core, win), is_hi)

    lo_T = np.maximum(1, -(-lo_cnt.max(axis=0) // P))
    hi_T = -(-hi_cnt.max(axis=0) // P)
    win_T = lo_T + hi_T
    T_total = int(win_T.sum())

    # Paired-window stream: [wA_lo | wB_lo | wA_hi | wB_hi] so one gather
    # call covers both windows' lo (resp. hi) tiles -> fewer, bigger calls.
    lo_base = np.zeros(W_WIN, np.int64)
    hi_base = np.zeros(W_WIN, np.int64)
    pairs = [tuple(range(p, min(p + 2, W_WIN))) for p in range(0, W_WIN, 2)]
    gcalls = []   # per pair: [(tile_offset, n_tiles, is_hi), ...]
    spans = {}    # window -> [(tile_offset, n_tiles), ...] for matmuls
    t = 0
    for pr in pairs:
        t0 = t
        for w in pr:
            lo_base[w] = t * P
            spans[w] = [(t, int(lo_T[w]))]
            t += int(lo_T[w])
        calls = [(t0, t - t0, 0)]
        t1 = t
        for w in pr:
            hi_base[w] = t * P
            if hi_T[w] > 0:
                spans[w].append((t, int(hi_T[w])))
            t += int(hi_T[w])
        if t > t1:
            calls.append((t1, t - t1, 1))
        gcalls.append(calls)
    assert t == T_total
    runs = None  # superseded by pairs/gcalls/spans

    idx_all = np.zeros((NCORES, T_total * P), np.int16)
    oh_val = np.zeros((NCORES, T_total * P), np.float32)
    oh_d8 = np.zeros((NCORES, T_total * P), np.int64)  # pads stay d8=0,val=0

    order = np.lexsort((is_hi, win, core))
    e_core, e_win, e_hi = core[order], win[order], is_hi[order]
    e_srcg, e_d8, e_val = srcg[order], d8[order], edge_val[order]

    grp = (e_core * W_WIN + e_win) * 2 + e_hi
    cnt = np.zeros(NCORES * W_WIN * 2 + 1, np.int64)
    np.add.at(cnt, grp + 1, 1)
    starts = np.cumsum(cnt)[:-1]
    within = np.arange(len(order)) - starts[grp]

    slot = np.where(e_hi == 0, lo_base[e_win], hi_base[e_win]) + within
    e_sc = e_srcg // NPAD
    e_ps = e_srcg % NPAD
    newidx = np.where(e_hi == 0, e_sc * ASH + e_ps,
                      e_sc * BSH + (e_ps - SPLIT))
    idx_all[e_core, slot] = newidx.astype(np.int16)
    oh_val[e_core, slot] = e_val
    oh_d8[e_core, slot] = e_d8

    return dict(idx_all=idx_all, oh_val=oh_val, oh_d8=oh_d8, pairs=pairs,
                gcalls=gcalls, spans=spans, win_T=win_T, T_total=T_total,
                perms=perms)


def build_in_maps(inputs, pp):
    import ml_dtypes
    raw_x = np.ascontiguousarray(inputs["raw_x"], dtype=np.float32)
    T = pp["T_total"]

    def wblocks(w, fout_pad):
        wp = np.zeros((w.shape[0], fout_pad), np.float32)
        wp[:, :w.shape[1]] = w
        kh = w.shape[0] // P
        return wp.reshape(kh, P, fout_pad)

    W012 = np.stack([
        np.stack([np.stack([inputs[k][a * P:(a + 1) * P, b * P:(b + 1) * P]
                            for b in range(2)]) for a in range(2)])
        for k in ("W_gc0", "W_gc1", "W_gc2")]).astype(ml_dtypes.bfloat16)
    W3 = wblocks(np.asarray(inputs["W_gc3"], np.float32),
                 128).astype(ml_dtypes.bfloat16)
    Wr0 = np.stack([np.stack([inputs["W_res0"][a * P:(a + 1) * P, b * P:(b + 1) * P]
                              for b in range(2)]) for a in range(2)]
                   ).astype(ml_dtypes.bfloat16)
    WrL = wblocks(np.asarray(inputs["W_res_last"], np.float32), 64)
    ident = np.eye(P, dtype=np.float32)

    in_maps = []
    for c in range(NCORES):
        xT0 = np.zeros((2, P, NPAD), np.float32)
        xloc = raw_x[c * NLOC:(c + 1) * NLOC]          # [NLOC, 256]
        xT0[:, :, pp["perms"][c]] = xloc.T.reshape(2, P, NLOC)
        xT0 = xT0.astype(ml_dtypes.bfloat16)

        idx16 = np.zeros((16, T * 8), np.int16)
        i = np.arange(T * P)
        idx16[i % 16, i // 16] = (
            (np.arange(T * P) % (NCORES * ASH)).astype(np.int16)
            if _os.environ.get("K_IDXSEQ") else pp["idx_all"][c])
        gidx = np.tile(idx16, (8, 1))                  # [128, T*8]

        # host-built one-hot tiles: oh[e, t, d] = val * (d == dest row), in
        # gather-slot-major order; layer-invariant, so built once and fp8.
        oh = np.zeros((T * P, P), np.float32)
        oh[np.arange(T * P), pp["oh_d8"][c]] = pp["oh_val"][c]
        ohtiles = np.ascontiguousarray(
            oh.reshape(T, P, P).transpose(1, 0, 2)).astype(
                ml_dtypes.float8_e4m3)

        in_maps.append(dict(xT0=xT0, gidx=gidx, ohtiles=ohtiles, W012=W012,
                            W3=W3, Wr0=Wr0, WrL=WrL, ident=ident))
    return in_maps


# ------------------------------------------------------------- device program


def build_program(pp, qmap=None):
    T = pp["T_total"]
    win_T = pp["win_T"]
    pairs, gcalls, spans = pp["pairs"], pp["gcalls"], pp["spans"]
    Tmax = max(sum(int(win_T[w]) for w in pr) for pr in pairs)
    NG = [(g * 512, min(512, NPAD - g * 512)) for g in range(-(-NPAD // 512))]

    nc = bacc.Bacc("TRN2", target_bir_lowering=False, debug=False,
                   num_devices=NCORES, num_swdge_queues=GATHER_QUEUES)

    xT0_d = nc.dram_tensor("xT0", [2, P, NPAD], BF16, kind="ExternalInput")
    gidx_d = nc.dram_tensor("gidx", [P, T * 8], I16, kind="ExternalInput")
    oh_d = nc.dram_tensor("ohtiles", [P, T, P], FP8, kind="ExternalInput")
    W012_d = nc.dram_tensor("W012", [3, 2, 2, P, P], BF16, kind="ExternalInput")
    W3_d = nc.dram_tensor("W3", [2, P, 128], BF16, kind="ExternalInput")
    Wr0_d = nc.dram_tensor("Wr0", [2, 2, P, P], BF16, kind="ExternalInput")
    WrL_d = nc.dram_tensor("WrL", [2, P, 64], F32, kind="ExternalInput")
    ident_d = nc.dram_tensor("ident", [P, P], F32, kind="ExternalInput")
    out_d = nc.dram_tensor("out", [NPAD, C_OUT], F32, kind="ExternalOutput")

    ag_inA = [nc.dram_tensor(f"ag_inA{i}", [ASH, HID], FP8) for i in range(2)]
    ag_inB = [nc.dram_tensor(f"ag_inB{i}", [BSH, HID], FP8) for i in range(2)]
    hfullA = [nc.dram_tensor(f"hfullA{i}", [NCORES * ASH, HID], FP8,
                             addr_space="Shared") for i in range(2)]
    hfullB = [nc.dram_tensor(f"hfullB{i}", [NCORES * BSH, HID], FP8,
                             addr_space="Shared") for i in range(2)]
    ag3A = nc.dram_tensor("ag3A", [ASH, 128], BF16)
    ag3B = nc.dram_tensor("ag3B", [BSH, 128], BF16)
    h3A = nc.dram_tensor("h3A", [NCORES * ASH, 128], BF16,
                         addr_space="Shared")
    h3B = nc.dram_tensor("h3B", [NCORES * BSH, 128], BF16,
                         addr_space="Shared")

    with tile.TileContext(nc) as tc:
        with (
            tc.tile_pool(name="pers", bufs=1) as pers,
            tc.tile_pool(name="wbufp", bufs=4) as wbufp,
            tc.tile_pool(name="ohp", bufs=4) as ohp,
            tc.tile_pool(name="hTs", bufs=3) as hTsp,
            tc.tile_pool(name="hns", bufs=4) as hnsp,
            tc.tile_pool(name="zs", bufs=4) as zsp,
            tc.tile_pool(name="r0s", bufs=4) as r0sp,
            tc.tile_pool(name="sm", bufs=6) as smp,
            tc.tile_pool(name="ps_s", bufs=4, space="PSUM") as ps_s,
            tc.tile_pool(name="ps_d", bufs=2, space="PSUM") as ps_d,
            tc.tile_pool(name="ps_t", bufs=2, space="PSUM") as ps_t,
        ):
            # ---------------- persistent loads
            xT = pers.tile([P, 2, NPAD], BF16, tag="xT")
            nc.sync.dma_start(out=xT[:, 0, :], in_=xT0_d.ap()[0])
            nc.sync.dma_start(out=xT[:, 1, :], in_=xT0_d.ap()[1])
            idxs = pers.tile([P, T * 8], I16, tag="gidx")
            nc.sync.dma_start(out=idxs[:], in_=gidx_d.ap())
            w012 = pers.tile([P, 12, P], BF16, tag="w012")
            nc.sync.dma_start(
                out=w012[:],
                in_=W012_d.ap().rearrange("a b c p m -> p (a b c) m"))
            w3 = pers.tile([P, 2, 128], BF16, tag="w3")
            nc.sync.dma_start(out=w3[:], in_=W3_d.ap().rearrange("a p m -> p a m"))
            wr0 = pers.tile([P, 4, P], BF16, tag="wr0")
            nc.sync.dma_start(out=wr0[:],
                              in_=Wr0_d.ap().rearrange("a b p m -> p (a b) m"))
            wrl = pers.tile([P, 2, 64], F32, tag="wrl")
            nc.sync.dma_start(out=wrl[:], in_=WrL_d.ap().rearrange("a p m -> p a m"))
            ident = pers.tile([P, P], F32, tag="ident")
            nc.sync.dma_start(out=ident[:], in_=ident_d.ap())
            # SBUF-resident residuals (window-indexed) + softmax staging
            res0_sb = pers.tile([P, W_WIN, HID], F32, tag="res0sb")
            rlast_sb = pers.tile([P, W_WIN, 64], F32, tag="rlastsb")
            tt_sb = pers.tile([P, W_WIN, C_OUT], F32, tag="ttsb")
            ssum_sb = pers.tile([P, W_WIN], F32, tag="ssumsb")

            def w012_ap(li, kh, fh):
                return w012[:, li * 4 + kh * 2 + fh, :]

            # ------------- dense helper: h[:, fh*128:...] = x @ W  (+ hooks)
            def dense256(w_ap_fn, dest_dram, rl_hook=None, hn_dt=BF16,
                         hn_tag="hn", mid_hook=None, dest_sb=None):
                """w_ap_fn(kh, fh) -> lhsT [128,128]. Writes node-major
                rows to dest_dram=(destA, destB) split at row SPLIT, or into
                dest_sb[:, w, :] (SBUF, window-indexed). mid_hook() runs
                after the last A group -- launches the A-shard AllGather."""
                destA, destB = (dest_dram if isinstance(dest_dram, tuple)
                                else (dest_dram, None))
                for g0, ng in NG:
                    stages = []
                    for fh in range(2):
                        psd = ps_d.tile([P, 512], F32, space="PSUM", tag="dps")
                        for kh in range(2):
                            nc.tensor.matmul(
                                psd[:, :ng], lhsT=w_ap_fn(kh, fh),
                                rhs=xT[:, kh, g0:g0 + ng],
                                start=(kh == 0), stop=(kh == 1))
                        hTst = hTsp.tile([P, 512], F32, tag="hT")
                        nc.vector.tensor_copy(out=hTst[:, :ng], in_=psd[:, :ng])
                        stages.append(hTst)
                        if rl_hook is not None:
                            rl_hook(fh, g0, ng, hTst)
                    for s in range(ng // P):
                        w = g0 // P + s
                        hn = (None if dest_sb is not None else
                              hnsp.tile([P, HID], hn_dt, tag=hn_tag))
                        for fh in range(2):
                            pst = ps_t.tile([P, P], F32, space="PSUM", tag="tp")
                            nc.tensor.transpose(
                                out=pst[:],
                                in_=stages[fh][:, s * P:(s + 1) * P],
                                identity=ident[:])
                            nc.vector.tensor_copy(
                                out=(dest_sb[:, w, fh * P:(fh + 1) * P]
                                     if dest_sb is not None
                                     else hn[:, fh * P:(fh + 1) * P]),
                                in_=pst[:])
                        if dest_sb is None:
                            r0 = g0 + s * P
                            dst = (destA.ap()[r0:r0 + P, :] if destB is None
                                   or r0 < SPLIT else
                                   destB.ap()[r0 - SPLIT:r0 - SPLIT + P, :])
                            nc.sync.dma_start(out=dst, in_=hn[:])
                    if mid_hook is not None and g0 + ng == SPLIT:
                        mid_hook()

            # ---------------- init: res0 (+ rlast) from raw_x
            rl_ps = {}

            def rl_hook(fh, g0, ng, hTst):
                if fh == 0:
                    rl_ps["t"] = ps_d.tile([P, 512], F32, space="PSUM",
                                           tag="dps", name="psr")
                psr = rl_ps["t"]
                nc.tensor.matmul(psr[:64, :ng], lhsT=wrl[:, fh, :],
                                 rhs=hTst[:, :ng],
                                 start=(fh == 0), stop=(fh == 1))
                if fh == 1:
                    rlT = hTsp.tile([P, 512], F32, tag="hT")
                    nc.vector.tensor_copy(out=rlT[:64, :ng], in_=psr[:64, :ng])
                    for s in range(ng // P):
                        w = g0 // P + s
                        pst = ps_t.tile([P, P], F32, space="PSUM", tag="tp")
                        nc.tensor.transpose(out=pst[:, :64],
                                            in_=rlT[:64, s * P:(s + 1) * P],
                                            identity=ident[:64, :64])
                        nc.vector.tensor_copy(out=rlast_sb[:, w, :],
                                              in_=pst[:, :64])

            # (res0/rlast dense is issued inside the layer-0 region below,
            # after the layer-0 AllGathers launch, so it runs while they fly)

            # ---------------- spmm helper (paired windows)
            qrr = [0]  # round-robin SWDGE queue counter
            gsems = [nc.alloc_semaphore(f"gsem{q}")
                     for q in range(GATHER_QUEUES)] if GATHER_PREP else None
            last_trig = [None] * GATHER_QUEUES
            prep_cnt = [0] * GATHER_QUEUES  # completed-DMA sem targets (/16)
            gnames = []  # emission-ordered gather instruction names
            owner = {}
            for w, sp in spans.items():
                for (t0, nt) in sp:
                    for t in range(t0, t0 + nt):
                        owner[t] = w

            def spmm(hfA, hfB, elem, psw, evict_fn, gdt=BF16):
                for ip, pr in enumerate(pairs):
                    p0 = spans[pr[0]][0][0]
                    nTp = sum(int(win_T[w]) for w in pr)
                    ohb = ohp.tile([P, Tmax, P], FP8, tag="ohb")
                    if SKIP_OH:
                        nc.vector.memset(ohb[:, 0, :], 0.0)
                    else:
                        # host-precomputed one-hot tiles; split the load so
                        # consecutive halves land on different DMA lanes
                        h = (nTp + 1) // 2
                        nc.sync.dma_start(out=ohb[:, 0:h, :],
                                          in_=oh_d.ap()[:, p0:p0 + h, :])
                        nc.sync.dma_start(
                            out=ohb[:, h:nTp, :],
                            in_=oh_d.ap()[:, p0 + h:p0 + nTp, :])
                    wbg = wbufp.tile([P, Tmax, elem], gdt, tag="wbg")
                    if SKIP_GATHER:
                        nc.vector.memset(wbg[:, 0, :], 0.0)
                    gwaits = {}  # tile idx -> [(sem, target, wait_names)]
                    for (c0, cn, hi) in gcalls[ip]:
                        if SKIP_GATHER:
                            continue
                        src = (hfB.ap() if hi else hfA.ap())
                        # one sub-call per SWDGE queue so each call-group's
                        # transfer drains through all four queue FIFOs
                        nsub = min(GATHER_NSUB, cn) or 1
                        bounds = [c0 + (cn * k) // nsub for k in range(nsub + 1)]
                        qs = set()
                        for k in range(nsub):
                            t0, t1 = bounds[k], bounds[k + 1]
                            nt = t1 - t0
                            if nt == 0:
                                continue
                            gi = qrr[0]
                            if GATHER_PREP:
                                q = gi % GATHER_QUEUES
                                # prep writes only descriptors; the transfer
                                # fires at the trigger below and completion
                                # bumps gsems[q] by 16
                                gin = nc.gpsimd.dma_gather(
                                    wbg[:, t0 - p0:t0 - p0 + nt, :], src,
                                    idxs[:, t0 * 8:(t0 + nt) * 8],
                                    nt * P, nt * P, elem,
                                    single_packet=GATHER_SINGLE_PACKET,
                                    queue_num=q, prepare_only=True,
                                    sem=gsems[q])
                                # pin ring order: a prep may not be scheduled
                                # across an earlier trigger of its queue, else
                                # that trigger would fire this prep's entries
                                if last_trig[q] is not None:
                                    dep = InstructionNameOrderedSet()
                                    dep.add(last_trig[q])
                                    gin.ins.add_nosync_dependencies_from(dep)
                                prep_cnt[q] += 1
                                qs.add(q)
                            else:
                                # queue must equal (scheduled DMASW lane) % 4;
                                # the scheduler can reorder gathers, so a
                                # corrective qmap takes precedence
                                q = (qmap[gi] if qmap is not None
                                     and gi < len(qmap)
                                     else gi % GATHER_QUEUES)
                                gin = nc.gpsimd.dma_gather(
                                    wbg[:, t0 - p0:t0 - p0 + nt, :], src,
                                    idxs[:, t0 * 8:(t0 + nt) * 8],
                                    nt * P, nt * P, elem,
                                    single_packet=GATHER_SINGLE_PACKET,
                                    queue_num=q)
                            gnames.append(gin.ins.name)
                            qrr[0] += 1
                        for q in sorted(qs):
                            trig = nc.gpsimd.trigger_dma(count=None,
                                                         queue_num=q)
                            last_trig[q] = trig.ins.name
                        if GATHER_PREP:
                            # PE must wait for this call-group's data before
                            # consuming tiles [c0, c0+cn)
                            gwaits[c0] = [(gsems[q], prep_cnt[q] * 16)
                                          for q in sorted(qs)]
                    # fp8 gather output feeds the matmul directly (mixed
                    # bf16 lhsT x fp8 rhs); when both operands are fp8,
                    # DoubleRow mode folds two consecutive same-window tiles
                    # into one PE instruction (256-deep contraction)
                    wb = wbg
                    dbl = DOUBLE_ROW and gdt == FP8 and not SKIP_SPMM_MM
                    psl = {w: ps_s.tile([P, HID], F32, space="PSUM",
                                        tag="sps", name=f"pss_{ip}_{w}")
                           for w in pr}
                    cnt = {w: 0 for w in pr}
                    wait_names = []

                    def mm(w, t, take):
                        tot = int(win_T[w])
                        if SKIP_SPMM_MM and not (cnt[w] == 0 or
                                                 cnt[w] + take >= tot):
                            cnt[w] += take
                            return
                        if take == 2:
                            mi = nc.tensor.matmul(
                                psl[w][:, :psw],
                                lhsT=ohb[:, t - p0:t - p0 + 2, :],
                                rhs=wb[:, t - p0:t - p0 + 2, :],
                                start=(cnt[w] == 0),
                                stop=(cnt[w] + 2 == tot),
                                perf_mode=mybir.MatmulPerfMode.DoubleRow)
                        else:
                            mi = nc.tensor.matmul(
                                psl[w][:, :psw],
                                lhsT=ohb[:, t - p0, :],
                                rhs=wb[:, t - p0, :],
                                start=(cnt[w] == 0),
                                stop=(cnt[w] + 1 == tot))
                        if wait_names:
                            # keep every consumer behind its group's PE waits
                            # (PE executes in order; pin the schedule order)
                            dep = InstructionNameOrderedSet()
                            for wn in wait_names:
                                dep.add(wn)
                            mi.ins.add_nosync_dependencies_from(dep)
                        cnt[w] += take

                    t = p0
                    while t < p0 + nTp:
                        w = owner[t]
                        if t in gwaits:
                            wait_names = []
                            for sem, tgt in gwaits[t]:
                                wi = nc.tensor.wait_ge(sem, tgt)
                                wait_names.append(wi.ins.name)
                        take = (2 if dbl and t + 1 < p0 + nTp
                                and owner[t + 1] == w and t + 1 not in gwaits
                                else 1)
                        mm(w, t, take)
                        t += take
                    for w in pr:
                        evict_fn(w, psl[w])

            # ---------------- GCN layers 0..2
            rg = [list(range(NCORES))]

            def ag(in_d, out_d):
                if not SKIP_COLL:
                    nc.gpsimd.collective_compute(
                        "AllGather", mybir.AluOpType.bypass,
                        replica_groups=rg, ins=[in_d.ap()], outs=[out_d.ap()])

            for li in range(3):
                pa = li % 2
                dense256(lambda kh, fh, li=li: w012_ap(li, kh, fh),
                         (ag_inA[pa], ag_inB[pa]), hn_dt=FP8,
                         mid_hook=lambda pa=pa: ag(ag_inA[pa], hfullA[pa]))
                ag(ag_inB[pa], hfullB[pa])
                if li == 0:
                    # res0/rlast dense overlaps the layer-0 AllGathers (it
                    # must finish before the first evict overwrites xT; the
                    # tile deps enforce that)
                    dense256(lambda kh, fh: wr0[:, kh * 2 + fh, :], None,
                             rl_hook=rl_hook, dest_sb=res0_sb)

                def evict_gc(w, pss):
                    z = zsp.tile([P, HID], F32, tag="z")
                    nc.vector.tensor_add(out=z[:], in0=pss[:],
                                         in1=res0_sb[:, w, :])
                    for fh in range(2):
                        pst = ps_t.tile([P, P], F32, space="PSUM", tag="tp")
                        nc.tensor.transpose(out=pst[:],
                                            in_=z[:, fh * P:(fh + 1) * P],
                                            identity=ident[:])
                        nc.scalar.activation(
                            out=xT[:, fh, w * P:(w + 1) * P], in_=pst[:],
                            func=mybir.ActivationFunctionType.Relu)

                spmm(hfullA[pa], hfullB[pa], HID, HID, evict_gc, gdt=FP8)

            # ---------------- layer 3 dense (fout=64 incl. padding)
            for g0, ng in NG:
                psd = ps_d.tile([P, 512], F32, space="PSUM", tag="dps")
                for kh in range(2):
                    nc.tensor.matmul(psd[:, :ng], lhsT=w3[:, kh, :],
                                     rhs=xT[:, kh, g0:g0 + ng],
                                     start=(kh == 0), stop=(kh == 1))
                h3T = hTsp.tile([P, 512], F32, tag="hT")
                nc.vector.tensor_copy(out=h3T[:, :ng], in_=psd[:, :ng])
                for s in range(ng // P):
                    pst = ps_t.tile([P, P], F32, space="PSUM", tag="tp")
                    nc.tensor.transpose(out=pst[:],
                                        in_=h3T[:, s * P:(s + 1) * P],
                                        identity=ident[:])
                    h3n = hnsp.tile([P, 128], BF16, tag="h3n")
                    nc.vector.tensor_copy(out=h3n[:], in_=pst[:])
                    r0 = g0 + s * P
                    dst3 = (ag3A.ap()[r0:r0 + P, :] if r0 < SPLIT
                            else ag3B.ap()[r0 - SPLIT:r0 - SPLIT + P, :])
                    nc.sync.dma_start(out=dst3, in_=h3n[:])
                if g0 + ng == SPLIT:
                    ag(ag3A, h3A)
            ag(ag3B, h3B)

            # ---------------- layer 3 spmm + rlast + log_softmax
            # Exp runs per window (accumulating row sums into ssum_sb); Ln
            # runs ONCE over [P, W_WIN] at the end -- no per-window
            # activation-table reloads.
            def evict_out(w, pss):
                y = zsp.tile([P, 64], F32, tag="y")
                nc.vector.tensor_add(out=y[:], in0=pss[:, :64],
                                     in1=rlast_sb[:, w, :])
                m = smp.tile([P, 1], F32, tag="m")
                nc.vector.tensor_reduce(out=m[:], in_=y[:, :C_OUT],
                                        axis=mybir.AxisListType.X,
                                        op=mybir.AluOpType.max)
                nc.vector.tensor_scalar(out=tt_sb[:, w, :], in0=y[:, :C_OUT],
                                        scalar1=m[:], scalar2=None,
                                        op0=mybir.AluOpType.subtract)
                e = smp.tile([P, C_OUT], F32, tag="e")
                nc.scalar.activation(out=e[:], in_=tt_sb[:, w, :],
                                     func=mybir.ActivationFunctionType.Exp,
                                     accum_out=ssum_sb[:, w:w + 1])

            spmm(h3A, h3B, 128, 128, evict_out)

            lgs = pers.tile([P, W_WIN], F32, tag="lgs")
            nc.scalar.activation(out=lgs[:], in_=ssum_sb[:],
                                 func=mybir.ActivationFunctionType.Ln)
            for w in range(W_WIN):
                o = smp.tile([P, C_OUT], F32, tag="o")
                nc.vector.tensor_scalar(out=o[:], in0=tt_sb[:, w, :],
                                        scalar1=lgs[:, w:w + 1], scalar2=None,
                                        op0=mybir.AluOpType.subtract)
                nc.sync.dma_start(out=out_d.ap()[w * P:(w + 1) * P, :],
                                  in_=o[:])

    nc.compile()
    nc._gather_names = gnames
    return nc


def _gather_qmap(nc):
    """Walk the scheduled program: gather g's DMASW sem lane is its
    scheduled Pool-DMA index %8, so its queue_num must be (lane %
    GATHER_QUEUES) or cross-queue sem races ensue. Returns (qmap keyed by
    emission order, already_aligned)."""
    import concourse.bass_isa as bass_isa
    sched_q = {}
    emitted_q = {}
    k = 0
    for blk in nc.m.functions[0].blocks:
        for inst in blk.instructions:
            if (inst.engine == mybir.EngineType.Pool
                    and isinstance(inst, bass_isa.AnyDMAInstruction)
                    and not isinstance(inst,
                                       bass_isa.UserSyncedRemoteDMADescs)):
                sched_q[inst.name] = (k % 8) % GATHER_QUEUES
                emitted_q[inst.name] = inst.queue_num
                k += 1
    qmap = [sched_q[nm] for nm in nc._gather_names]
    ok = all(sched_q[nm] == emitted_q[nm] for nm in nc._gather_names)
    return qmap, ok


def _build_aligned(pp):
    """Build, then realign gather queue_nums to the scheduled sem lanes
    (the tile scheduler may reorder gathers); iterate to a fixpoint.
    In GATHER_PREP mode each queue has its own explicit DMA semaphore, so
    no realignment is needed."""
    nc = build_program(pp)
    if GATHER_PREP:
        return nc
    for _ in range(3):
        qmap, ok = _gather_qmap(nc)
        if ok:
            return nc
        nc = build_program(pp, qmap)
    qmap, ok = _gather_qmap(nc)
    assert ok, "gather queue/sem-lane alignment did not converge"
    return nc


# ------------------------------------------------------------ timed benchmark


def bench(nc, in_maps, iters=8):
    """Times repeated on-device executions with device-resident inputs
    (replicates bass2jax.run_bass_via_pjrt's multi-core path). Returns
    (best_seconds, per_iter_seconds)."""
    import time

    import jax
    from jax.experimental.shard_map import shard_map
    from jax.sharding import Mesh, NamedSharding, PartitionSpec

    from concourse import bass2jax, mybir as mb

    bass2jax.install_neuronx_cc_hook()

    partition_name = (nc.partition_id_tensor.name
                      if nc.partition_id_tensor else None)
    in_names, out_names, out_avals, zero_outs = [], [], [], []
    for alloc in nc.m.functions[0].allocations:
        if not isinstance(alloc, mb.MemoryLocationSet):
            continue
        name = alloc.memorylocations[0].name
        if alloc.kind == "ExternalInput":
            if name != partition_name:
                in_names.append(name)
        elif alloc.kind == "ExternalOutput":
            out_names.append(name)
            shape = tuple(alloc.tensor_shape)
            dtype = mb.dt.np(alloc.dtype)
            out_avals.append(jax.core.ShapedArray(shape, dtype))
            zero_outs.append(np.zeros(shape, dtype))
    n_params = len(in_names)
    n_outs = len(out_avals)
    all_names = in_names + out_names
    if partition_name is not None:
        all_names = all_names + [partition_name]

    def _body(*args):
        operands = list(args)
        if partition_name is not None:
            operands.append(bass2jax.partition_id_tensor())
        outs = bass2jax._bass_exec_p.bind(
            *operands, out_avals=tuple(out_avals), in_names=tuple(all_names),
            out_names=tuple(out_names), lowering_input_output_aliases=(),
            sim_require_finite=True, sim_require_nnan=True, nc=nc)
        return tuple(outs)

    devices = jax.devices()[:NCORES]
    mesh = Mesh(np.asarray(devices), ("core",))
    in_specs = (PartitionSpec("core"),) * (n_params + n_outs)
    out_specs = (PartitionSpec("core"),) * n_outs
    donate = tuple(range(n_params, n_params + n_outs))
    del donate
    sharded = jax.jit(shard_map(_body, mesh=mesh, in_specs=in_specs,
                                out_specs=out_specs, check_rep=False),
                      keep_unused=True)

    sh = NamedSharding(mesh, PartitionSpec("core"))
    dev_in = [
        jax.device_put(
            np.concatenate([np.asarray(in_maps[c][n]) for c in range(NCORES)],
                           axis=0), sh)
        for n in in_names]
    zglobal = [np.zeros((NCORES * z.shape[0], *z.shape[1:]), z.dtype)
               for z in zero_outs]

    dz = [jax.device_put(z, sh) for z in zglobal]
    for d in dz:
        d.block_until_ready()

    def run_batch(n):
        """Queue n executions without intermediate sync; return elapsed."""
        t0 = time.perf_counter()
        outs = None
        for _ in range(n):
            outs = sharded(*dev_in, *dz)
        for o in outs:
            o.block_until_ready()
        return time.perf_counter() - t0

    run_batch(2)  # warmup
    n_lo, n_hi = 6, 46
    reps = max(3, iters // 2)
    lo = min(run_batch(n_lo) for _ in range(reps))
    hi = min(run_batch(n_hi) for _ in range(reps))
    per_exec = (hi - lo) / (n_hi - n_lo)
    return per_exec, (lo, hi)


# ---------------------------------------------------------------- entry point

_CACHE = {}


def _run(inputs, trace=False, trace_kwargs=None):
    pp = preprocess(np.asarray(inputs["edge_row"]),
                    np.asarray(inputs["edge_col"]),
                    np.asarray(inputs["edge_val"], dtype=np.float32))
    in_maps = build_in_maps(inputs, pp)
    key = (pp["T_total"], tuple(tuple(c) for cs in pp["gcalls"] for c in cs))
    if key not in _CACHE:
        _CACHE.clear()
        _CACHE[key] = _build_aligned(pp)
    nc = _CACHE[key]
    res = run_bass_kernel_spmd(nc, in_maps, list(range(NCORES)), trace=trace,
                               **(trace_kwargs or {}))
    outs = [res.results[c]["out"][pp["perms"][c]] for c in range(NCORES)]
    full = np.concatenate(outs, axis=0).astype(np.float32)
    return full, res


def kernel(**inputs):
    out, _ = _run(inputs)
    return out



# revision 29
# speedup vs baseline: 1.4522x; 1.4265x over previous
"""DeepGCN ResNet (4-layer GCN w/ residuals + log_softmax) on 8 TRN2 NeuronCores.

Sharding: nodes split 8 ways; edges partitioned by destination row.
Per layer: local dense x@W (bf16) -> two fp8 AllGathers of h, split by
source-position bucket (A = rows < SPLIT per core, B = rest); the A-shard
collective launches mid-dense and A-bucket gathers overlap the B collective.
Source rows are fetched per edge with dma_gather (fp8, 256B descriptors,
single_packet, 4 SWDGE queues so all four Q7 descriptor-generation pairs run
concurrently), multiplied by HOST-precomputed fp8 one-hot tiles (DMA-loaded
from DRAM per window pair; the graph is layer-invariant so the tiles are
built once on host — this keeps the DVE out of the inner loop entirely) on
TensorE with PSUM accumulation per 128-dest window, then +res0
(SBUF-resident) and relu.
The res0/rlast projection itself runs under the layer-0 AllGather shadow.
Final layer adds res0@W_res_last and a batched log_softmax (one Ln pass).

Hard-won constraints (see memory notes): gather queue_num MUST equal the
scheduled DMASW sem-lane %4 (_build_aligned enforces this; the tile
scheduler reorders gathers); single_packet caps calls at 64 descs/engine
(<=5 tiles); pad gather indices with 0, never -1.

Host preprocessing is layout-only: greedy dest->window packing, edge
bucketing/sorting, int16 index packing (per-bucket index spaces), and the
per-tile (dest-row, val) scalar table for the DVE one-hot build.
"""

import numpy as np

import concourse.bacc as bacc
import concourse.mybir as mybir
import concourse.tile as tile
from concourse.bass import InstructionNameOrderedSet
from concourse.bass_utils import run_bass_kernel_spmd

P = 128

# Problem geometry (hardcoded per the task contract).
N_NODES = 50000
N_EDGES = 800000
F_IN = 256
HID = 256
C_OUT = 40
NCORES = 8

NLOC = N_NODES // NCORES            # 6250
NPAD = ((NLOC + P - 1) // P) * P    # 6272
W_WIN = NPAD // P                   # 49 windows of 128 dest rows
# Source rows are bucketed by their (permuted) position within the owner
# core: A = rows [0, SPLIT), B = rows [SPLIT, NPAD). Each bucket gets its own
# AllGather, so A-tile gathers can run while the B AllGather is in flight.
# Both bucket index spaces stay under int16 max.
# A-bucket rows per core; 28*P=3584 keeps 8*ASH=28672 within int16 and the
# larger A share gives the B-AllGather less exposed latency at dense end
SPLIT = 28 * P                      # 3584 (7 dense groups of 512)
ASH = SPLIT                         # A shard rows per core
BSH = NPAD - SPLIT                  # 3200 B shard rows per core


def _set_geometry(n_nodes):
    """Debug hook: shrink the node count (keeps F/HID/C). Used only by the
    small-scale simulator test, never in grading."""
    global N_NODES, NLOC, NPAD, W_WIN, SPLIT, ASH, BSH
    N_NODES = n_nodes
    NLOC = N_NODES // NCORES
    NPAD = ((NLOC + P - 1) // P) * P
    W_WIN = NPAD // P
    SPLIT = (W_WIN // 2) * P
    ASH = SPLIT
    BSH = NPAD - SPLIT

F32 = mybir.dt.float32
BF16 = mybir.dt.bfloat16
FP8 = mybir.dt.float8e4
I16 = mybir.dt.int16
MAX_GATHER_TILES = 999
import os as _os
# single-packet descriptors measured best (traced exec 2018us vs 2210us for
# multi-packet); K_NOSP=1 switches to multi-packet mode
GATHER_SINGLE_PACKET = not _os.environ.get("K_NOSP")
# prepare_only/trigger mode (K_PREP=1): measured SLOWER (3.42ms vs 2.28ms) --
# the framework does not defer a DRAM-source dep to the trigger, so the prep
# still waits on the AllGather and the added triggers/PE-waits are pure
# overhead. Kept for reference.
GATHER_PREP = bool(_os.environ.get("K_PREP"))
# fp8 DoubleRow: one PE instruction per two same-window edge tiles
DOUBLE_ROW = not _os.environ.get("K_NODBL")
SKIP_GATHER = bool(_os.environ.get("K_SKIP_GATHER"))   # bisect: omit dma_gather
SKIP_SPMM_MM = bool(_os.environ.get("K_SKIP_SPMM"))    # bisect: omit segment matmuls
SKIP_COLL = bool(_os.environ.get("K_SKIP_COLL"))       # bisect: omit AllGathers
SKIP_OH = bool(_os.environ.get("K_SKIP_OH"))           # bisect: omit oh tile loads
GATHER_QUEUES = int(_os.environ.get("K_GQ", "4"))  # rotate dma_gather queues
# sub-calls per gather call (desc-gen fixed-cost amortization); defaults to
# GATHER_QUEUES (legacy: one sub-call per queue)
GATHER_NSUB = int(_os.environ.get("K_NSUB", "0")) or GATHER_QUEUES


# ----------------------------------------------------------------- host prep


def preprocess(edge_row, edge_col, edge_val):
    """Edge partitioning/sorting/padding with per-core dest->window
    rebalancing (greedy 2D bin-pack on lo/hi in-degree). Returns per-core
    data arrays plus a core-uniform tile structure."""
    edge_row = edge_row.astype(np.int64)
    edge_col = edge_col.astype(np.int64)

    # --- lo/hi in-degree per destination (lo = src owned by cores 0..3)
    src_core0 = edge_col // NLOC
    src_hi0 = src_core0 >= (NCORES // 2)
    lo_deg = np.zeros(N_NODES, np.int64)
    hi_deg = np.zeros(N_NODES, np.int64)
    np.add.at(lo_deg, edge_row, ~src_hi0)
    np.add.at(hi_deg, edge_row, src_hi0)

    # --- per-core greedy assignment of dests to windows (balance lo & hi)
    pos_of_node = np.zeros(N_NODES, np.int64)
    perms = []  # per core: padded_pos[d_local]
    for c in range(NCORES):
        ld = lo_deg[c * NLOC:(c + 1) * NLOC].astype(np.float64)
        hd = hi_deg[c * NLOC:(c + 1) * NLOC].astype(np.float64)
        order = np.argsort(-(ld + hd), kind="stable")
        # windows 0..N_BIG-1 get a 9-tile budget (1152); rest hard-capped at
        # 1024 so they stay 8 tiles. Every core's excess lands in the same
        # window indices, so the cross-core max stays tight.
        N_BIG = 5
        cap = np.full(W_WIN, 8.0 * P)
        cap[:N_BIG] = 9.0 * P
        loads_lo = np.zeros(W_WIN)
        loads_hi = np.zeros(W_WIN)
        counts = np.zeros(W_WIN, np.int64)
        wassign = np.zeros(NLOC, np.int64)
        for t, d in enumerate(order):
            ccap = min(P, t // W_WIN + 2)  # stay within 2 of even fill
            cost = np.maximum(loads_lo + ld[d], loads_hi + hd[d])
            infeas = ((counts >= ccap) | (loads_lo + ld[d] > cap)
                      | (loads_hi + hd[d] > cap))
            if infeas.all():
                infeas = counts >= ccap
            if infeas.all():
                infeas = counts >= P
            cost = np.where(infeas, 1e18, cost)
            w = int(np.argmin(cost))
            wassign[d] = w
            loads_lo[w] += ld[d]
            loads_hi[w] += hd[d]
            counts[w] += 1
        # slot within window
        slot_in_w = np.zeros(NLOC, np.int64)
        fill = np.zeros(W_WIN, np.int64)
        for d in range(NLOC):
            w = wassign[d]
            slot_in_w[d] = fill[w]
            fill[w] += 1
        p = wassign * P + slot_in_w
        perms.append(p)
        pos_of_node[c * NLOC:(c + 1) * NLOC] = c * NPAD + p

    core = edge_row // NLOC
    p_local = pos_of_node[edge_row] - core * NPAD
    win = p_local // P
    d8 = p_local % P
    srcg = pos_of_node[edge_col]
    p_src = srcg % NPAD
    is_hi = (p_src >= SPLIT).astype(np.int64)  # B bucket

    lo_cnt = np.zeros((NCORES, W_WIN), np.int64)
    hi_cnt = np.zeros((NCORES, W_WIN), np.int64)
    np.add.at(lo_cnt, (core, win), 1 - is_hi)
    np.add.at(hi_cnt, (core, win), is_hi)

    lo_T = np.maximum(1, -(-lo_cnt.max(axis=0) // P))
    hi_T = -(-hi_cnt.max(axis=0) // P)
    win_T = lo_T + hi_T
    T_total = int(win_T.sum())

    # Paired-window stream: [wA_lo | wB_lo | wA_hi | wB_hi] so one gather
    # call covers both windows' lo (resp. hi) tiles -> fewer, bigger calls.
    lo_base = np.zeros(W_WIN, np.int64)
    hi_base = np.zeros(W_WIN, np.int64)
    pairs = [tuple(range(p, min(p + 2, W_WIN))) for p in range(0, W_WIN, 2)]
    gcalls = []   # per pair: [(tile_offset, n_tiles, is_hi), ...]
    spans = {}    # window -> [(tile_offset, n_tiles), ...] for matmuls
    t = 0
    for pr in pairs:
        t0 = t
        for w in pr:
            lo_base[w] = t * P
            spans[w] = [(t, int(lo_T[w]))]
            t += int(lo_T[w])
        calls = [(t0, t - t0, 0)]
        t1 = t
        for w in pr:
            hi_base[w] = t * P
            if hi_T[w] > 0:
                spans[w].append((t, int(hi_T[w])))
            t += int(hi_T[w])
        if t > t1:
            calls.append((t1, t - t1, 1))
        gcalls.append(calls)
    assert t == T_total
    runs = None  # superseded by pairs/gcalls/spans

    idx_all = np.zeros((NCORES, T_total * P), np.int16)
    oh_val = np.zeros((NCORES, T_total * P), np.float32)
    oh_d8 = np.zeros((NCORES, T_total * P), np.int64)  # pads stay d8=0,val=0

    order = np.lexsort((is_hi, win, core))
    e_core, e_win, e_hi = core[order], win[order], is_hi[order]
    e_srcg, e_d8, e_val = srcg[order], d8[order], edge_val[order]

    grp = (e_core * W_WIN + e_win) * 2 + e_hi
    cnt = np.zeros(NCORES * W_WIN * 2 + 1, np.int64)
    np.add.at(cnt, grp + 1, 1)
    starts = np.cumsum(cnt)[:-1]
    within = np.arange(len(order)) - starts[grp]

    slot = np.where(e_hi == 0, lo_base[e_win], hi_base[e_win]) + within
    e_sc = e_srcg // NPAD
    e_ps = e_srcg % NPAD
    newidx = np.where(e_hi == 0, e_sc * ASH + e_ps,
                      e_sc * BSH + (e_ps - SPLIT))
    idx_all[e_core, slot] = newidx.astype(np.int16)
    oh_val[e_core, slot] = e_val
    oh_d8[e_core, slot] = e_d8

    return dict(idx_all=idx_all, oh_val=oh_val, oh_d8=oh_d8, pairs=pairs,
                gcalls=gcalls, spans=spans, win_T=win_T, T_total=T_total,
                perms=perms)


def build_in_maps(inputs, pp):
    import ml_dtypes
    raw_x = np.ascontiguousarray(inputs["raw_x"], dtype=np.float32)
    T = pp["T_total"]

    def wblocks(w, fout_pad):
        wp = np.zeros((w.shape[0], fout_pad), np.float32)
        wp[:, :w.shape[1]] = w
        kh = w.shape[0] // P
        return wp.reshape(kh, P, fout_pad)

    W012 = np.stack([
        np.stack([np.stack([inputs[k][a * P:(a + 1) * P, b * P:(b + 1) * P]
                            for b in range(2)]) for a in range(2)])
        for k in ("W_gc0", "W_gc1", "W_gc2")]).astype(ml_dtypes.bfloat16)
    W3 = wblocks(np.asarray(inputs["W_gc3"], np.float32),
                 128).astype(ml_dtypes.bfloat16)
    Wr0 = np.stack([np.stack([inputs["W_res0"][a * P:(a + 1) * P, b * P:(b + 1) * P]
                              for b in range(2)]) for a in range(2)]
                   ).astype(ml_dtypes.bfloat16)
    WrL = wblocks(np.asarray(inputs["W_res_last"], np.float32), 64)
    ident = np.eye(P, dtype=np.float32)

    in_maps = []
    for c in range(NCORES):
        xT0 = np.zeros((2, P, NPAD), np.float32)
        xloc = raw_x[c * NLOC:(c + 1) * NLOC]          # [NLOC, 256]
        xT0[:, :, pp["perms"][c]] = xloc.T.reshape(2, P, NLOC)
        xT0 = xT0.astype(ml_dtypes.bfloat16)

        idx16 = np.zeros((16, T * 8), np.int16)
        i = np.arange(T * P)
        idx16[i % 16, i // 16] = (
            (np.arange(T * P) % (NCORES * ASH)).astype(np.int16)
            if _os.environ.get("K_IDXSEQ") else pp["idx_all"][c])
        gidx = np.tile(idx16, (8, 1))                  # [128, T*8]

        # host-built one-hot tiles: oh[e, t, d] = val * (d == dest row), in
        # gather-slot-major order; layer-invariant, so built once and fp8.
        oh = np.zeros((T * P, P), np.float32)
        oh[np.arange(T * P), pp["oh_d8"][c]] = pp["oh_val"][c]
        ohtiles = np.ascontiguousarray(
            oh.reshape(T, P, P).transpose(1, 0, 2)).astype(
                ml_dtypes.float8_e4m3)

        in_maps.append(dict(xT0=xT0, gidx=gidx, ohtiles=ohtiles, W012=W012,
                            W3=W3, Wr0=Wr0, WrL=WrL, ident=ident))
    return in_maps


# ------------------------------------------------------------- device program


def build_program(pp, qmap=None):
    T = pp["T_total"]
    win_T = pp["win_T"]
    pairs, gcalls, spans = pp["pairs"], pp["gcalls"], pp["spans"]
    Tmax = max(sum(int(win_T[w]) for w in pr) for pr in pairs)
    NG = [(g * 512, min(512, NPAD - g * 512)) for g in range(-(-NPAD // 512))]

    nc = bacc.Bacc("TRN2", target_bir_lowering=False, debug=False,
                   num_devices=NCORES, num_swdge_queues=GATHER_QUEUES)

    xT0_d = nc.dram_tensor("xT0", [2, P, NPAD], BF16, kind="ExternalInput")
    gidx_d = nc.dram_tensor("gidx", [P, T * 8], I16, kind="ExternalInput")
    oh_d = nc.dram_tensor("ohtiles", [P, T, P], FP8, kind="ExternalInput")
    W012_d = nc.dram_tensor("W012", [3, 2, 2, P, P], BF16, kind="ExternalInput")
    W3_d = nc.dram_tensor("W3", [2, P, 128], BF16, kind="ExternalInput")
    Wr0_d = nc.dram_tensor("Wr0", [2, 2, P, P], BF16, kind="ExternalInput")
    WrL_d = nc.dram_tensor("WrL", [2, P, 64], F32, kind="ExternalInput")
    ident_d = nc.dram_tensor("ident", [P, P], F32, kind="ExternalInput")
    out_d = nc.dram_tensor("out", [NPAD, C_OUT], F32, kind="ExternalOutput")

    ag_inA = [nc.dram_tensor(f"ag_inA{i}", [ASH, HID], FP8) for i in range(2)]
    ag_inB = [nc.dram_tensor(f"ag_inB{i}", [BSH, HID], FP8) for i in range(2)]
    hfullA = [nc.dram_tensor(f"hfullA{i}", [NCORES * ASH, HID], FP8,
                             addr_space="Shared") for i in range(2)]
    hfullB = [nc.dram_tensor(f"hfullB{i}", [NCORES * BSH, HID], FP8,
                             addr_space="Shared") for i in range(2)]
    ag3A = nc.dram_tensor("ag3A", [ASH, 128], BF16)
    ag3B = nc.dram_tensor("ag3B", [BSH, 128], BF16)
    h3A = nc.dram_tensor("h3A", [NCORES * ASH, 128], BF16,
                         addr_space="Shared")
    h3B = nc.dram_tensor("h3B", [NCORES * BSH, 128], BF16,
                         addr_space="Shared")

    with tile.TileContext(nc) as tc:
        with (
            tc.tile_pool(name="pers", bufs=1) as pers,
            tc.tile_pool(name="wbufp", bufs=5) as wbufp,
            tc.tile_pool(name="ohp", bufs=6) as ohp,
            tc.tile_pool(name="hTs", bufs=4) as hTsp,
            tc.tile_pool(name="hns", bufs=4) as hnsp,
            tc.tile_pool(name="zs", bufs=4) as zsp,
            tc.tile_pool(name="r0s", bufs=4) as r0sp,
            tc.tile_pool(name="sm", bufs=6) as smp,
            tc.tile_pool(name="ps_s", bufs=4, space="PSUM") as ps_s,
            tc.tile_pool(name="ps_d", bufs=2, space="PSUM") as ps_d,
            tc.tile_pool(name="ps_t", bufs=2, space="PSUM") as ps_t,
        ):
            # ---------------- persistent loads
            xT = pers.tile([P, 2, NPAD], BF16, tag="xT")
            nc.sync.dma_start(out=xT[:, 0, :], in_=xT0_d.ap()[0])
            nc.sync.dma_start(out=xT[:, 1, :], in_=xT0_d.ap()[1])
            idxs = pers.tile([P, T * 8], I16, tag="gidx")
            nc.sync.dma_start(out=idxs[:], in_=gidx_d.ap())
            w012 = pers.tile([P, 12, P], BF16, tag="w012")
            nc.sync.dma_start(
                out=w012[:],
                in_=W012_d.ap().rearrange("a b c p m -> p (a b c) m"))
            w3 = pers.tile([P, 2, 128], BF16, tag="w3")
            nc.sync.dma_start(out=w3[:], in_=W3_d.ap().rearrange("a p m -> p a m"))
            wr0 = pers.tile([P, 4, P], BF16, tag="wr0")
            nc.sync.dma_start(out=wr0[:],
                              in_=Wr0_d.ap().rearrange("a b p m -> p (a b) m"))
            wrl = pers.tile([P, 2, 64], F32, tag="wrl")
            nc.sync.dma_start(out=wrl[:], in_=WrL_d.ap().rearrange("a p m -> p a m"))
            ident = pers.tile([P, P], F32, tag="ident")
            nc.sync.dma_start(out=ident[:], in_=ident_d.ap())
            # SBUF-resident residuals (window-indexed) + softmax staging
            res0_sb = pers.tile([P, W_WIN, HID], BF16, tag="res0sb")
            rlast_sb = pers.tile([P, W_WIN, 64], F32, tag="rlastsb")
            tt_sb = pers.tile([P, W_WIN, C_OUT], F32, tag="ttsb")
            ssum_sb = pers.tile([P, W_WIN], F32, tag="ssumsb")

            def w012_ap(li, kh, fh):
                return w012[:, li * 4 + kh * 2 + fh, :]

            # ------------- dense helper: h[:, fh*128:...] = x @ W  (+ hooks)
            def dense256(w_ap_fn, dest_dram, rl_hook=None, hn_dt=BF16,
                         hn_tag="hn", mid_hook=None, dest_sb=None):
                """w_ap_fn(kh, fh) -> lhsT [128,128]. Writes node-major
                rows to dest_dram=(destA, destB) split at row SPLIT, or into
                dest_sb[:, w, :] (SBUF, window-indexed). mid_hook() runs
                after the last A group -- launches the A-shard AllGather."""
                destA, destB = (dest_dram if isinstance(dest_dram, tuple)
                                else (dest_dram, None))
                for g0, ng in NG:
                    stages = []
                    for fh in range(2):
                        psd = ps_d.tile([P, 512], F32, space="PSUM", tag="dps")
                        for kh in range(2):
                            nc.tensor.matmul(
                                psd[:, :ng], lhsT=w_ap_fn(kh, fh),
                                rhs=xT[:, kh, g0:g0 + ng],
                                start=(kh == 0), stop=(kh == 1))
                        hTst = hTsp.tile([P, 512], F32, tag="hT")
                        nc.vector.tensor_copy(out=hTst[:, :ng], in_=psd[:, :ng])
                        stages.append(hTst)
                        if rl_hook is not None:
                            rl_hook(fh, g0, ng, hTst)
                    for s in range(ng // P):
                        w = g0 // P + s
                        hn = (None if dest_sb is not None else
                              hnsp.tile([P, HID], hn_dt, tag=hn_tag))
                        for fh in range(2):
                            pst = ps_t.tile([P, P], F32, space="PSUM", tag="tp")
                            nc.tensor.transpose(
                                out=pst[:],
                                in_=stages[fh][:, s * P:(s + 1) * P],
                                identity=ident[:])
                            nc.vector.tensor_copy(
                                out=(dest_sb[:, w, fh * P:(fh + 1) * P]
                                     if dest_sb is not None
                                     else hn[:, fh * P:(fh + 1) * P]),
                                in_=pst[:])
                        if dest_sb is None:
                            r0 = g0 + s * P
                            dst = (destA.ap()[r0:r0 + P, :] if destB is None
                                   or r0 < SPLIT else
                                   destB.ap()[r0 - SPLIT:r0 - SPLIT + P, :])
                            nc.sync.dma_start(out=dst, in_=hn[:])
                    if mid_hook is not None and g0 + ng == SPLIT:
                        mid_hook()

            # ---------------- init: res0 (+ rlast) from raw_x
            rl_ps = {}

            def rl_hook(fh, g0, ng, hTst):
                if fh == 0:
                    rl_ps["t"] = ps_d.tile([P, 512], F32, space="PSUM",
                                           tag="dps", name="psr")
                psr = rl_ps["t"]
                nc.tensor.matmul(psr[:64, :ng], lhsT=wrl[:, fh, :],
                                 rhs=hTst[:, :ng],
                                 start=(fh == 0), stop=(fh == 1))
                if fh == 1:
                    rlT = hTsp.tile([P, 512], F32, tag="hT")
                    nc.vector.tensor_copy(out=rlT[:64, :ng], in_=psr[:64, :ng])
                    for s in range(ng // P):
                        w = g0 // P + s
                        pst = ps_t.tile([P, P], F32, space="PSUM", tag="tp")
                        nc.tensor.transpose(out=pst[:, :64],
                                            in_=rlT[:64, s * P:(s + 1) * P],
                                            identity=ident[:64, :64])
                        nc.vector.tensor_copy(out=rlast_sb[:, w, :],
                                              in_=pst[:, :64])

            # (res0/rlast dense is issued inside the layer-0 region below,
            # after the layer-0 AllGathers launch, so it runs while they fly)

            # ---------------- spmm helper (paired windows)
            qrr = [0]  # round-robin SWDGE queue counter
            gsems = [nc.alloc_semaphore(f"gsem{q}")
                     for q in range(GATHER_QUEUES)] if GATHER_PREP else None
            last_trig = [None] * GATHER_QUEUES
            prep_cnt = [0] * GATHER_QUEUES  # completed-DMA sem targets (/16)
            gnames = []  # emission-ordered gather instruction names
            owner = {}
            for w, sp in spans.items():
                for (t0, nt) in sp:
                    for t in range(t0, t0 + nt):
                        owner[t] = w

            def spmm(hfA, hfB, elem, psw, evict_fn, gdt=BF16):
                for ip, pr in enumerate(pairs):
                    p0 = spans[pr[0]][0][0]
                    nTp = sum(int(win_T[w]) for w in pr)
                    ohb = ohp.tile([P, Tmax, P], FP8, tag="ohb")
                    if SKIP_OH:
                        nc.vector.memset(ohb[:, 0, :], 0.0)
                    else:
                        # host-precomputed one-hot tiles; split the load so
                        # consecutive halves land on different DMA lanes
                        h = (nTp + 1) // 2
                        nc.sync.dma_start(out=ohb[:, 0:h, :],
                                          in_=oh_d.ap()[:, p0:p0 + h, :])
                        nc.sync.dma_start(
                            out=ohb[:, h:nTp, :],
                            in_=oh_d.ap()[:, p0 + h:p0 + nTp, :])
                    wbg = wbufp.tile([P, Tmax, elem], gdt, tag="wbg")
                    if SKIP_GATHER:
                        nc.vector.memset(wbg[:, 0, :], 0.0)
                    gwaits = {}  # tile idx -> [(sem, target, wait_names)]
                    for (c0, cn, hi) in gcalls[ip]:
                        if SKIP_GATHER:
                            continue
                        src = (hfB.ap() if hi else hfA.ap())
                        # one sub-call per SWDGE queue so each call-group's
                        # transfer drains through all four queue FIFOs
                        nsub = min(GATHER_NSUB, cn) or 1
                        bounds = [c0 + (cn * k) // nsub for k in range(nsub + 1)]
                        qs = set()
                        for k in range(nsub):
                            t0, t1 = bounds[k], bounds[k + 1]
                            nt = t1 - t0
                            if nt == 0:
                                continue
                            gi = qrr[0]
                            if GATHER_PREP:
                                q = gi % GATHER_QUEUES
                                # prep writes only descriptors; the transfer
                                # fires at the trigger below and completion
                                # bumps gsems[q] by 16
                                gin = nc.gpsimd.dma_gather(
                                    wbg[:, t0 - p0:t0 - p0 + nt, :], src,
                                    idxs[:, t0 * 8:(t0 + nt) * 8],
                                    nt * P, nt * P, elem,
                                    single_packet=GATHER_SINGLE_PACKET,
                                    queue_num=q, prepare_only=True,
                                    sem=gsems[q])
                                # pin ring order: a prep may not be scheduled
                                # across an earlier trigger of its queue, else
                                # that trigger would fire this prep's entries
                                if last_trig[q] is not None:
                                    dep = InstructionNameOrderedSet()
                                    dep.add(last_trig[q])
                                    gin.ins.add_nosync_dependencies_from(dep)
                                prep_cnt[q] += 1
                                qs.add(q)
                            else:
                                # queue must equal (scheduled DMASW lane) % 4;
                                # the scheduler can reorder gathers, so a
                                # corrective qmap takes precedence
                                q = (qmap[gi] if qmap is not None
                                     and gi < len(qmap)
                                     else gi % GATHER_QUEUES)
                                gin = nc.gpsimd.dma_gather(
                                    wbg[:, t0 - p0:t0 - p0 + nt, :], src,
                                    idxs[:, t0 * 8:(t0 + nt) * 8],
                                    nt * P, nt * P, elem,
                                    single_packet=GATHER_SINGLE_PACKET,
                                    queue_num=q)
                            gnames.append(gin.ins.name)
                            qrr[0] += 1
                        for q in sorted(qs):
                            trig = nc.gpsimd.trigger_dma(count=None,
                                                         queue_num=q)
                            last_trig[q] = trig.ins.name
                        if GATHER_PREP:
                            # PE must wait for this call-group's data before
                            # consuming tiles [c0, c0+cn)
                            gwaits[c0] = [(gsems[q], prep_cnt[q] * 16)
                                          for q in sorted(qs)]
                    # fp8 gather output feeds the matmul directly (mixed
                    # bf16 lhsT x fp8 rhs); when both operands are fp8,
                    # DoubleRow mode folds two consecutive same-window tiles
                    # into one PE instruction (256-deep contraction)
                    wb = wbg
                    dbl = DOUBLE_ROW and gdt == FP8 and not SKIP_SPMM_MM
                    psl = {w: ps_s.tile([P, HID], F32, space="PSUM",
                                        tag="sps", name=f"pss_{ip}_{w}")
                           for w in pr}
                    cnt = {w: 0 for w in pr}
                    wait_names = []

                    def mm(w, t, take):
                        tot = int(win_T[w])
                        if SKIP_SPMM_MM and not (cnt[w] == 0 or
                                                 cnt[w] + take >= tot):
                            cnt[w] += take
                            return
                        if take == 2:
                            mi = nc.tensor.matmul(
                                psl[w][:, :psw],
                                lhsT=ohb[:, t - p0:t - p0 + 2, :],
                                rhs=wb[:, t - p0:t - p0 + 2, :],
                                start=(cnt[w] == 0),
                                stop=(cnt[w] + 2 == tot),
                                perf_mode=mybir.MatmulPerfMode.DoubleRow)
                        else:
                            mi = nc.tensor.matmul(
                                psl[w][:, :psw],
                                lhsT=ohb[:, t - p0, :],
                                rhs=wb[:, t - p0, :],
                                start=(cnt[w] == 0),
                                stop=(cnt[w] + 1 == tot))
                        if wait_names:
                            # keep every consumer behind its group's PE waits
                            # (PE executes in order; pin the schedule order)
                            dep = InstructionNameOrderedSet()
                            for wn in wait_names:
                                dep.add(wn)
                            mi.ins.add_nosync_dependencies_from(dep)
                        cnt[w] += take

                    t = p0
                    while t < p0 + nTp:
                        w = owner[t]
                        if t in gwaits:
                            wait_names = []
                            for sem, tgt in gwaits[t]:
                                wi = nc.tensor.wait_ge(sem, tgt)
                                wait_names.append(wi.ins.name)
                        take = (2 if dbl and t + 1 < p0 + nTp
                                and owner[t + 1] == w and t + 1 not in gwaits
                                else 1)
                        mm(w, t, take)
                        t += take
                    for w in pr:
                        evict_fn(w, psl[w])

            # ---------------- GCN layers 0..2
            rg = [list(range(NCORES))]

            def ag(in_d, out_d):
                if not SKIP_COLL:
                    nc.gpsimd.collective_compute(
                        "AllGather", mybir.AluOpType.bypass,
                        replica_groups=rg, ins=[in_d.ap()], outs=[out_d.ap()])

            for li in range(3):
                pa = li % 2
                dense256(lambda kh, fh, li=li: w012_ap(li, kh, fh),
                         (ag_inA[pa], ag_inB[pa]), hn_dt=FP8,
                         mid_hook=lambda pa=pa: ag(ag_inA[pa], hfullA[pa]))
                ag(ag_inB[pa], hfullB[pa])
                if li == 0:
                    # res0/rlast dense overlaps the layer-0 AllGathers (it
                    # must finish before the first evict overwrites xT; the
                    # tile deps enforce that)
                    dense256(lambda kh, fh: wr0[:, kh * 2 + fh, :], None,
                             rl_hook=rl_hook, dest_sb=res0_sb)

                def evict_gc(w, pss):
                    z = zsp.tile([P, HID], F32, tag="z")
                    nc.vector.tensor_add(out=z[:], in0=pss[:],
                                         in1=res0_sb[:, w, :])
                    for fh in range(2):
                        pst = ps_t.tile([P, P], F32, space="PSUM", tag="tp")
                        nc.tensor.transpose(out=pst[:],
                                            in_=z[:, fh * P:(fh + 1) * P],
                                            identity=ident[:])
                        nc.scalar.activation(
                            out=xT[:, fh, w * P:(w + 1) * P], in_=pst[:],
                            func=mybir.ActivationFunctionType.Relu)

                spmm(hfullA[pa], hfullB[pa], HID, HID, evict_gc, gdt=FP8)

            # ---------------- layer 3 dense (fout=64 incl. padding)
            for g0, ng in NG:
                psd = ps_d.tile([P, 512], F32, space="PSUM", tag="dps")
                for kh in range(2):
                    nc.tensor.matmul(psd[:, :ng], lhsT=w3[:, kh, :],
                                     rhs=xT[:, kh, g0:g0 + ng],
                                     start=(kh == 0), stop=(kh == 1))
                h3T = hTsp.tile([P, 512], F32, tag="hT")
                nc.vector.tensor_copy(out=h3T[:, :ng], in_=psd[:, :ng])
                for s in range(ng // P):
                    pst = ps_t.tile([P, P], F32, space="PSUM", tag="tp")
                    nc.tensor.transpose(out=pst[:],
                                        in_=h3T[:, s * P:(s + 1) * P],
                                        identity=ident[:])
                    h3n = hnsp.tile([P, 128], BF16, tag="h3n")
                    nc.vector.tensor_copy(out=h3n[:], in_=pst[:])
                    r0 = g0 + s * P
                    dst3 = (ag3A.ap()[r0:r0 + P, :] if r0 < SPLIT
                            else ag3B.ap()[r0 - SPLIT:r0 - SPLIT + P, :])
                    nc.sync.dma_start(out=dst3, in_=h3n[:])
                if g0 + ng == SPLIT:
                    ag(ag3A, h3A)
            ag(ag3B, h3B)

            # ---------------- layer 3 spmm + rlast + log_softmax
            # Exp runs per window (accumulating row sums into ssum_sb); Ln
            # runs ONCE over [P, W_WIN] at the end -- no per-window
            # activation-table reloads.
            def evict_out(w, pss):
                y = zsp.tile([P, 64], F32, tag="y")
                nc.vector.tensor_add(out=y[:], in0=pss[:, :64],
                                     in1=rlast_sb[:, w, :])
                m = smp.tile([P, 1], F32, tag="m")
                nc.vector.tensor_reduce(out=m[:], in_=y[:, :C_OUT],
                                        axis=mybir.AxisListType.X,
                                        op=mybir.AluOpType.max)
                nc.vector.tensor_scalar(out=tt_sb[:, w, :], in0=y[:, :C_OUT],
                                        scalar1=m[:], scalar2=None,
                                        op0=mybir.AluOpType.subtract)
                e = smp.tile([P, C_OUT], F32, tag="e")
                nc.scalar.activation(out=e[:], in_=tt_sb[:, w, :],
                                     func=mybir.ActivationFunctionType.Exp,
                                     accum_out=ssum_sb[:, w:w + 1])

            spmm(h3A, h3B, 128, 128, evict_out)

            lgs = pers.tile([P, W_WIN], F32, tag="lgs")
            nc.scalar.activation(out=lgs[:], in_=ssum_sb[:],
                                 func=mybir.ActivationFunctionType.Ln)
            for w in range(W_WIN):
                o = smp.tile([P, C_OUT], F32, tag="o")
                nc.vector.tensor_scalar(out=o[:], in0=tt_sb[:, w, :],
                                        scalar1=lgs[:, w:w + 1], scalar2=None,
                                        op0=mybir.AluOpType.subtract)
                nc.sync.dma_start(out=out_d.ap()[w * P:(w + 1) * P, :],
                                  in_=o[:])

    nc.compile()
    nc._gather_names = gnames
    return nc


def _gather_qmap(nc):
    """Walk the scheduled program: gather g's DMASW sem lane is its
    scheduled Pool-DMA index %8, so its queue_num must be (lane %
    GATHER_QUEUES) or cross-queue sem races ensue. Returns (qmap keyed by
    emission order, already_aligned)."""
    import concourse.bass_isa as bass_isa
    sched_q = {}
    emitted_q = {}
    k = 0
    for blk in nc.m.functions[0].blocks:
        for inst in blk.instructions:
            if (inst.engine == mybir.EngineType.Pool
                    and isinstance(inst, bass_isa.AnyDMAInstruction)
                    and not isinstance(inst,
                                       bass_isa.UserSyncedRemoteDMADescs)):
                sched_q[inst.name] = (k % 8) % GATHER_QUEUES
                emitted_q[inst.name] = inst.queue_num
                k += 1
    qmap = [sched_q[nm] for nm in nc._gather_names]
    ok = all(sched_q[nm] == emitted_q[nm] for nm in nc._gather_names)
    return qmap, ok


def _build_aligned(pp):
    """Build, then realign gather queue_nums to the scheduled sem lanes
    (the tile scheduler may reorder gathers); iterate to a fixpoint.
    In GATHER_PREP mode each queue has its own explicit DMA semaphore, so
    no realignment is needed."""
    nc = build_program(pp)
    if GATHER_PREP:
        return nc
    for _ in range(3):
        qmap, ok = _gather_qmap(nc)
        if ok:
            return nc
        nc = build_program(pp, qmap)
    qmap, ok = _gather_qmap(nc)
    assert ok, "gather queue/sem-lane alignment did not converge"
    return nc


# ------------------------------------------------------------ timed benchmark


def bench(nc, in_maps, iters=8):
    """Times repeated on-device executions with device-resident inputs
    (replicates bass2jax.run_bass_via_pjrt's multi-core path). Returns
    (best_seconds, per_iter_seconds)."""
    import time

    import jax
    from jax.experimental.shard_map import shard_map
    from jax.sharding import Mesh, NamedSharding, PartitionSpec

    from concourse import bass2jax, mybir as mb

    bass2jax.install_neuronx_cc_hook()

    partition_name = (nc.partition_id_tensor.name
                      if nc.partition_id_tensor else None)
    in_names, out_names, out_avals, zero_outs = [], [], [], []
    for alloc in nc.m.functions[0].allocations:
        if not isinstance(alloc, mb.MemoryLocationSet):
            continue
        name = alloc.memorylocations[0].name
        if alloc.kind == "ExternalInput":
            if name != partition_name:
                in_names.append(name)
        elif alloc.kind == "ExternalOutput":
            out_names.append(name)
            shape = tuple(alloc.tensor_shape)
            dtype = mb.dt.np(alloc.dtype)
            out_avals.append(jax.core.ShapedArray(shape, dtype))
            zero_outs.append(np.zeros(shape, dtype))
    n_params = len(in_names)
    n_outs = len(out_avals)
    all_names = in_names + out_names
    if partition_name is not None:
        all_names = all_names + [partition_name]

    def _body(*args):
        operands = list(args)
        if partition_name is not None:
            operands.append(bass2jax.partition_id_tensor())
        outs = bass2jax._bass_exec_p.bind(
            *operands, out_avals=tuple(out_avals), in_names=tuple(all_names),
            out_names=tuple(out_names), lowering_input_output_aliases=(),
            sim_require_finite=True, sim_require_nnan=True, nc=nc)
        return tuple(outs)

    devices = jax.devices()[:NCORES]
    mesh = Mesh(np.asarray(devices), ("core",))
    in_specs = (PartitionSpec("core"),) * (n_params + n_outs)
    out_specs = (PartitionSpec("core"),) * n_outs
    donate = tuple(range(n_params, n_params + n_outs))
    del donate
    sharded = jax.jit(shard_map(_body, mesh=mesh, in_specs=in_specs,
                                out_specs=out_specs, check_rep=False),
                      keep_unused=True)

    sh = NamedSharding(mesh, PartitionSpec("core"))
    dev_in = [
        jax.device_put(
            np.concatenate([np.asarray(in_maps[c][n]) for c in range(NCORES)],
                           axis=0), sh)
        for n in in_names]
    zglobal = [np.zeros((NCORES * z.shape[0], *z.shape[1:]), z.dtype)
               for z in zero_outs]

    dz = [jax.device_put(z, sh) for z in zglobal]
    for d in dz:
        d.block_until_ready()

    def run_batch(n):
        """Queue n executions without intermediate sync; return elapsed."""
        t0 = time.perf_counter()
        outs = None
        for _ in range(n):
            outs = sharded(*dev_in, *dz)
        for o in outs:
            o.block_until_ready()
        return time.perf_counter() - t0

    import gc

    run_batch(2)  # warmup
    n_lo, n_hi = 6, 86
    reps = max(6, iters)
    gc.disable()
    try:
        lo = min(run_batch(n_lo) for _ in range(reps))
        hi = min(run_batch(n_hi) for _ in range(reps))
    finally:
        gc.enable()
    per_exec = (hi - lo) / (n_hi - n_lo)
    return per_exec, (lo, hi)


# ---------------------------------------------------------------- entry point

_CACHE = {}


def _run(inputs, trace=False, trace_kwargs=None):
    pp = preprocess(np.asarray(inputs["edge_row"]),
                    np.asarray(inputs["edge_col"]),
                    np.asarray(inputs["edge_val"], dtype=np.float32))
    in_maps = build_in_maps(inputs, pp)
    key = (pp["T_total"], tuple(tuple(c) for cs in pp["gcalls"] for c in cs))
    if key not in _CACHE:
        _CACHE.clear()
        _CACHE[key] = _build_aligned(pp)
    nc = _CACHE[key]
    res = run_bass_kernel_spmd(nc, in_maps, list(range(NCORES)), trace=trace,
                               **(trace_kwargs or {}))
    outs = [res.results[c]["out"][pp["perms"][c]] for c in range(NCORES)]
    full = np.concatenate(outs, axis=0).astype(np.float32)
    return full, res


def kernel(**inputs):
    out, _ = _run(inputs)
    return out

